# revision 76
# baseline (speedup 1.0000x reference)
# DeepSeek block (MLA attention + top-2-of-8 MoE + shared expert) on 8 TRN2
# NeuronCores, zero-collective sharding.
#
# Core c in [0..8): sequence b = c//4, q = c%4; owns token chunks
# hi = 7-q (slot 0) and lo = q (slot 1), 256 tokens each (causally balanced:
# every core's true causal work is 9 key blocks of 256).
#
# The SPMD program is identical on all cores; everything core-specific
# arrives as data: slot 0 attends key blocks [0..16), slot 1 [0..8), with
# host-built multiplicative masks (ones for fully-past blocks, triangular at
# the causal boundary, zeros for fully-future blocks).
#
# Layout: activations live feature-on-partition ("^T", tokens on the free
# axis). Host pre-transposes x, folds w_ln1/w_ln2 into adjacent weights,
# pre-casts weights to bf16, and builds rope tables / masks / identity.
# Matmuls run in bf16 (fp32 PSUM accumulate); the gate runs in fp32.
import os
import numpy as np
import ml_dtypes

import concourse.bacc as bacc
import concourse.mybir as mybir
import concourse.tile as tile
from concourse import bass_utils

F32 = mybir.dt.float32
F32R = mybir.dt.float32r
BF16 = mybir.dt.bfloat16
F8 = mybir.dt.float8e4
DR = mybir.MatmulPerfMode.DoubleRow
AF = mybir.ActivationFunctionType
ALU = mybir.AluOpType

WSCALE = 64.0            # fp8 expert-weight scale (keeps values normal-range)
HSCALE = 16.0            # fp8 h-activation scale (folded into comb weights)

B, T, C, H, D = 2, 2048, 1024, 16, 64
R, ROPE, NOPE = 128, 32, 32
E, I = 8, 512
THETA, EPS = 100000.0, 1e-5
P = 128
NCB = C // P             # 8 C blocks
NTB = T // P             # 16 key/token blocks per sequence
TLOC, CHUNK = 512, 256
KB_SLOT = (16, 8)        # key blocks attended per chunk slot
NKB = sum(KB_SLOT)
NIB = I // P             # 4 I blocks

DEBUG = bool(int(os.environ.get("BASSK_DEBUG", "0")))
_CACHE = {}


# =============================================================== device IR
def _emit(nc, tc):
    import contextlib

    def din(name, shape, dt):
        return nc.dram_tensor(name, shape, dt, kind="ExternalInput")

    xT8    = din("xT8", (C, T), F8)
    xlocT  = din("xlocT", (C, TLOC), F32)
    xloc8  = din("xloc8", (C, TLOC), F8)
    wq8    = din("wq8", (P, 4 * 2 * H * D), F8)    # DoubleRow pairs, x WSCALE
    wkva8  = din("wkva8", (P, 4 * 2 * (R + ROPE)), F8)
    wkvb   = din("wkvb", (R, H * NOPE), BF16)
    wo8    = din("wo8", (P, 2 * 2 * C), F8)   # DR pairs, x WSCALE
    cosq   = din("cosq", (2 * ROPE, TLOC), BF16)
    ssinq  = din("ssinq", (2 * ROPE, TLOC), BF16)
    coskp  = din("coskp", (ROPE, 4 * 512), BF16)  # [32, nt*512+t]
    ssinkp = din("ssinkp", (ROPE, 4 * 512), BF16)
    perm64 = din("perm64", (2 * ROPE, 2 * ROPE), BF16)
    perm32 = din("perm32", (ROPE, ROPE), BF16)
    ident  = din("ident", (P, P), F32)
    identbf = din("identbf", (P, P), BF16)
    sel8   = din("sel8", (E, E * P), BF16)   # sel8[k, e*128+m] = (k == e)
    kmask  = din("kmask", (P, NKB * CHUNK), BF16)
    wgate  = din("wgate", (C, E), F32)
    biasg  = din("biasg", (P, E), F32)
    # fp8 expert weights, expert 0 = shared.  wg8/wu8 rows e*P+k hold
    # [dcb, j, m] -> W[256*dcb + 128*j + k, m] * WSCALE (DoubleRow pairs on j).
    # wd8 rows e*P+k hold [dib, j, c] -> Wd[256*dib + 128*j + k, c] * WSCALE.
    wg8    = din("wg8", ((E + 1) * P, 4 * 2 * I), F8)
    wu8    = din("wu8", ((E + 1) * P, 4 * 2 * I), F8)
    wd8    = din("wd8", ((E + 1) * P, 2 * 2 * C), F8)

    outT = nc.dram_tensor("outT", (C, TLOC), F32, kind="ExternalOutput")
    dbg = {}
    if DEBUG:
        for name, shape in [("d_xaT", (C, TLOC)), ("d_comb", (P, 4 * E)),
                            ("d_invr1", (1, T)), ("d_invr2", (1, TLOC))]:
            dbg[name] = nc.dram_tensor(name, shape, F32, kind="ExternalOutput")

    f32r = lambda ap: ap.bitcast(F32R)

    # ---------------- pools (sized to fit 192KB/partition SBUF, 8 PSUM banks)
    whole = contextlib.ExitStack()   # whole kernel
    attn  = contextlib.ExitStack()   # until gate done
    early = contextlib.ExitStack()   # until Q/ckv/krope done
    xtst  = contextlib.ExitStack()   # xt tiles, until ckv done
    pc   = whole.enter_context(tc.tile_pool(name="pc", bufs=1))
    pq1  = attn.enter_context(tc.tile_pool(name="pq1", bufs=1))
    pkv0 = attn.enter_context(tc.tile_pool(name="pkv0", bufs=1))
    pps  = attn.enter_context(tc.tile_pool(name="psA", bufs=2, space="PSUM"))
    pacc = attn.enter_context(tc.tile_pool(name="psB", bufs=2, space="PSUM"))
    px   = early.enter_context(tc.tile_pool(name="px", bufs=1, side="right"))
    ptmp1 = early.enter_context(tc.tile_pool(name="tmp1", bufs=2, side="right"))
    pxt  = xtst.enter_context(tc.tile_pool(name="pxt", bufs=1, side="right"))

    # ---- constants
    ones_128x1 = pc.tile([P, 1], BF16); nc.any.memset(ones_128x1[:], 1.0)
    ones1f = pc.tile([1, 1], F32); nc.any.memset(ones1f[:], 1.0)
    eps_sb = pc.tile([1, 1], F32); nc.any.memset(eps_sb[:], EPS)
    eps64 = pc.tile([1, 1], F32)
    nc.any.memset(eps64[:], EPS * WSCALE * WSCALE)
    ident_sb = pc.tile([P, P], F32); nc.sync.dma_start(ident_sb[:], ident.ap())
    ident_bf = pc.tile([P, P], BF16)
    nc.sync.dma_start(ident_bf[:], identbf.ap())
    sel8_sb = pc.tile([E, E * P], BF16)
    nc.sync.dma_start(sel8_sb[:], sel8.ap())
    perm64_sb = pc.tile([2 * ROPE, 2 * ROPE], BF16)
    nc.sync.dma_start(perm64_sb[:], perm64.ap())
    perm32_sb = pc.tile([ROPE, ROPE], BF16)
    nc.sync.dma_start(perm32_sb[:], perm32.ap())
    biasg_sb = pc.tile([P, E], F32); nc.sync.dma_start(biasg_sb[:], biasg.ap())
    cosq_sb = px.tile([2 * ROPE, TLOC], BF16)
    nc.sync.dma_start(cosq_sb[:], cosq.ap())
    ssinq_sb = px.tile([2 * ROPE, TLOC], BF16)
    nc.sync.dma_start(ssinq_sb[:], ssinq.ap())
    coskp_all = px.tile([ROPE, 4, 512], BF16, name="coskp_all")
    nc.sync.dma_start(coskp_all[:], coskp.ap())
    coskp_sb = [coskp_all[:, nt, :] for nt in range(4)]
    ssinkp_all = px.tile([ROPE, 4, 512], BF16, name="ssinkp_all")
    nc.gpsimd.dma_start(ssinkp_all[:], ssinkp.ap())
    ssinkp_sb = [ssinkp_all[:, nt, :] for nt in range(4)]

    # ---- bulk loads (early), DMA issues spread across engine queues
    qeng = [nc.sync, nc.scalar, nc.sync, nc.gpsimd, nc.sync, nc.scalar]
    def load_blocks(pool, name, dram_ap, nblk, width, dt, nsplit, qoff=0):
        # one [P, nblk, width] tile, DMA'd in nsplit chunks across queues
        tl = pool.tile([P, nblk, width], dt, name=name)
        per = nblk // nsplit
        src_v = dram_ap.rearrange("(cb p) f -> p cb f", p=P)
        for s in range(nsplit):
            qeng[(qoff + s) % 6].dma_start(
                tl[:, s * per:(s + 1) * per, :],
                src_v[:, s * per:(s + 1) * per, :])
        return [tl[:, cb, :] for cb in range(nblk)]

    xt_t = pxt.tile([P, NCB, T], F8, name="xt8_all")
    for s in range(NCB):
        qeng[s % 6].dma_start(
            xt_t[:, s:s + 1, :],
            xT8.ap().rearrange("(cb p) f -> p cb f", p=P)[:, s:s + 1, :])
    xt = [xt_t[:, cb, :] for cb in range(NCB)]
    xloc = load_blocks(pq1, "xloc_all", xlocT.ap(), NCB, TLOC, F32, 2, 4)
    xl8_t = px.tile([P, NCB, TLOC], F8, name="xl8_all")
    nc.scalar.dma_start(xl8_t[:],
                        xloc8.ap().rearrange("(cb p) f -> p cb f", p=P))
    wq8_sb = px.tile([P, 4, 2, H * D], F8, name="wq8_sb")
    for s in range(2):
        qeng[s].dma_start(wq8_sb[:, 2 * s:2 * s + 2, :, :],
                          wq8.ap().rearrange("p (d j f) -> p d j f", d=4, j=2)
                          [:, 2 * s:2 * s + 2, :, :])
    wkva8_sb = px.tile([P, 4, 2, R + ROPE], F8, name="wkva8_sb")
    nc.gpsimd.dma_start(wkva8_sb[:],
                        wkva8.ap().rearrange("p (d j f) -> p d j f", d=4, j=2))
    wkvb_sb = pkv0.tile([R, H * NOPE], BF16, name="wkvbs")
    nc.gpsimd.dma_start(wkvb_sb[:], wkvb.ap())

    # ---- rmsnorm1 stats: global (keys) then local (queries)
    invr1 = px.tile([1, T], BF16)
    bc1 = px.tile([P, T], BF16)
    for nt in range(T // 512):
        sl = slice(nt * 512, (nt + 1) * 512)
        sps = pacc.tile([1, 512], F32, name="ssq1", tag="accA")
        for cb in range(NCB):
            xq = ptmp1.tile([P, 512], BF16, name="xsq", tag="xsq")
            if cb % 2 == 0:
                nc.scalar.activation(xq[:], xt[cb][:, sl], AF.Square)
            else:
                nc.vector.tensor_tensor(xq[:], xt[cb][:, sl], xt[cb][:, sl],
                                        ALU.mult)
            nc.tensor.matmul(sps[:], ones_128x1[:], xq[:],
                             start=(cb == 0), stop=(cb == NCB - 1))
        rr = ptmp1.tile([1, 512], F32, name="rms1", tag="rms1")
        nc.scalar.activation(rr[:], sps[:], AF.Sqrt, bias=eps64[:],
                             scale=WSCALE * WSCALE / C)
        with nc.allow_low_precision(reason="rmsnorm scale in bf16"):
            nc.vector.reciprocal(invr1[:, sl], rr[:])
        nc.gpsimd.partition_broadcast(bc1[:, sl], invr1[:, sl])


    # ---- ckv (scaled);  k_rope raw kept packed in two (64,512) tiles
    kvlat = pkv0.tile([R, T], BF16)
    kropef = [px.tile([ROPE, 512], BF16, name=f"kropef{i}") for i in range(4)]
    for nt in range(T // 512):
        sl = slice(nt * 512, (nt + 1) * 512)
        lat_ps = pps.tile([P, 512], F32, name="latps", tag="sps", bufs=2)
        rop_ps = pps.tile([ROPE, 512], F32, name="ropps", tag="sps", bufs=2)
        for dcb in range(4):
            nc.tensor.matmul(lat_ps[:], wkva8_sb[:, dcb, :, 0:R],
                             xt_t[:, 2 * dcb:2 * dcb + 2, sl],
                             start=(dcb == 0), stop=(dcb == 3), perf_mode=DR)
        for dcb in range(4):
            nc.tensor.matmul(rop_ps[:], wkva8_sb[:, dcb, :, R:R + ROPE],
                             xt_t[:, 2 * dcb:2 * dcb + 2, sl],
                             start=(dcb == 0), stop=(dcb == 3), perf_mode=DR)
        nc.vector.tensor_tensor(kvlat[:, sl], lat_ps[:], bc1[:, sl], ALU.mult)
        nc.vector.tensor_tensor(kropef[nt][:], rop_ps[:],
                                bc1[0:ROPE, sl], ALU.mult)

    invr1l = px.tile([1, TLOC], F32)
    bc1l = px.tile([P, TLOC], F32)
    spsl = pacc.tile([1, TLOC], F32, name="ssql", tag="accA")
    for cb in range(NCB):
        xq = ptmp1.tile([P, TLOC], BF16, name="xsql", tag="xsq")
        nc.scalar.activation(xq[:], xloc[cb][:], AF.Square)
        nc.tensor.matmul(spsl[:], ones_128x1[:], xq[:],
                         start=(cb == 0), stop=(cb == NCB - 1))
    rrl = ptmp1.tile([1, TLOC], F32, name="rmsl", tag="rms1")
    nc.scalar.activation(rrl[:], spsl[:], AF.Sqrt, bias=eps64[:],
                         scale=WSCALE * WSCALE / C)
    nc.vector.reciprocal(invr1l[:], rrl[:])
    nc.gpsimd.partition_broadcast(bc1l[:], invr1l[:])

    # ---- rope K -> kr32 (32, T)
    kr32 = pkv0.tile([ROPE, T], BF16)
    for nt in range(T // 512):
        sl = slice(nt * 512, (nt + 1) * 512)
        src = kropef[nt][:]
        par_ps = pps.tile([ROPE, 512], F32, name="parps", tag="sps", bufs=2)
        nc.tensor.matmul(par_ps[:], perm32_sb[:], src)
        t1 = ptmp1.tile([ROPE, 512], BF16, name="kr1", tag="kr1")
        nc.vector.tensor_tensor(t1[:], src, coskp_sb[nt][:], ALU.mult)
        t2 = ptmp1.tile([ROPE, 512], BF16, name="kr2", tag="kr2")
        nc.vector.tensor_tensor(t2[:], par_ps[:], ssinkp_sb[nt][:], ALU.mult)
        nc.vector.tensor_tensor(kr32[:, sl], t1[:], t2[:], ALU.add)

    xtst.close()
    pkv  = attn.enter_context(tc.tile_pool(name="pkv", bufs=1))
    pq2  = attn.enter_context(tc.tile_pool(name="pq2", bufs=1))
    ptmp2 = attn.enter_context(tc.tile_pool(name="tmp2", bufs=2))
    pE   = attn.enter_context(tc.tile_pool(name="pE", bufs=4))

    # ---- k_nope (transient), V_ext, kfull (emitted before Q so the PE/Act/
    # DMA work here overlaps the DVE-heavy Q phase)
    sub = contextlib.ExitStack()
    pkx = sub.enter_context(tc.tile_pool(name="pkx", bufs=2))
    kfull = [pkv.tile([P, T], BF16, name=f"kfull{mb}") for mb in range(8)]
    for half in range(2):
        knope = []
        for mb in (2 * half, 2 * half + 1):
            tl = pkx.tile([P, T], BF16, name="knope", tag="kn")
            for nt in range(T // 512):
                sl = slice(nt * 512, (nt + 1) * 512)
                ps = pps.tile([P, 512], F32, name="knps", tag="sps", bufs=2)
                nc.tensor.matmul(ps[:], wkvb_sb[:, mb * P:(mb + 1) * P],
                                 kvlat[:, sl])
                if nt % 2 == 0:
                    nc.scalar.copy(tl[:, sl], ps[:])
                else:
                    nc.vector.tensor_scalar(tl[:, sl], ps[:], 0.0, None,
                                            op0=ALU.add)
            knope.append(tl)
        for mb in range(4 * half, 4 * half + 4):
            tl = kfull[mb]
            h0, h1 = 2 * mb, 2 * mb + 1
            e0, e1 = (nc.sync, nc.gpsimd) if mb % 2 else (nc.gpsimd, nc.sync)
            e0.dma_start(tl[0:32, :],
                         knope[h0 // 4 - 2 * half]
                         [(h0 % 4) * 32:(h0 % 4) * 32 + 32, :])
            e1.dma_start(tl[64:96, :],
                         knope[h1 // 4 - 2 * half]
                         [(h1 % 4) * 32:(h1 % 4) * 32 + 32, :])
            e0.dma_start(tl[32:64, :], kr32[:])
            e1.dma_start(tl[96:128, :], kr32[:])
    sub.close()
    vext = [pkv.tile([P, H, 34], BF16, name=f"vext{tb}")
            for tb in range(NTB)]

    def emit_vext(tbs):
        for tb in tbs:
            tl = vext[tb]
            ps = pps.tile([P, H * NOPE], F32, name="vps", tag="sps", bufs=2)
            nc.tensor.matmul(ps[:], kvlat[:, tb * P:(tb + 1) * P], wkvb_sb[:])
            if tb % 2 == 0:
                nc.scalar.copy(tl[:, :, 0:NOPE],
                               ps[:].rearrange("p (h d) -> p h d", h=H))
            else:
                nc.vector.tensor_scalar(
                    tl[:, :, 0:NOPE],
                    ps[:].rearrange("p (h d) -> p h d", h=H),
                    0.0, None, op0=ALU.add)
            nc.any.memset(tl[:, :, NOPE:NOPE + 1], 1.0)

    # deferred loads for the scores/Wo/MoE phases (DMAs overlap attention)
    kmask_sb = pkv.tile([P, 16 * CHUNK], BF16)
    nc.gpsimd.dma_start(kmask_sb[:], kmask.ap()[:, 8 * CHUNK:])
    wo8_sb = pq2.tile([P, 2, 2, C], F8, name="wo8_sb")
    nc.sync.dma_start(wo8_sb[:],
                      wo8.ap().rearrange("p (d j f) -> p d j f", d=2, j=2))
    wgate_sb = pq2.tile([P, NCB, E], F32, name="wgate_sb")
    nc.gpsimd.dma_start(wgate_sb[:], wgate.ap().rearrange(
        "(cb p) e -> p cb e", p=P))

    # ---- attention core defs (q-major AV; exp in merged 512-wide tiles)
    # slot0 (q cols 0:256) attends kb 0..16, slot1 (cols 256:512) kb 0..8.
    # kb<8: one (128,512) score tile covers both slots; slot0 half needs no
    # mask (always fully-past), slot1 half gets kmask.  kb>=8 (slot0 only):
    # pairs (8+2p, 9+2p) share a (128,512) tile, masked as one 512-wide mult.
    # AV accumulates q-major: yq[:, qb*33:qb*33+33] = sum_k em[k,q] vext[k,:].
    yall = [pq2.tile([P, 2, TLOC], F8, name=f"yall{d}") for d in range(2)]
    yt4 = {}
    NO1 = NOPE + 1

    def av(yq, qb, kb, lhsT, h, start, stop):
        nc.tensor.matmul(yq[:, qb * NO1:(qb + 1) * NO1], lhsT,
                         vext[kb][:, h, 0:NO1], start=start, stop=stop)

    def attn_head(h):
        mb, po = h // 2, (h % 2) * 64
        yq = pacc.tile([P, 4 * NO1], F32, name="yq", tag="accA")
        for dp in range(4):
            s_ps = pps.tile([P, 2 * TLOC], F32, name="sps", tag="sps", bufs=2)
            for u in range(2):
                db = 2 * dp + u
                nc.tensor.matmul(s_ps[:, u * TLOC:(u + 1) * TLOC],
                                 kfull[mb][po:po + 64, db * P:(db + 1) * P],
                                 qbf[mb][po:po + 64, :])
            ee = pE.tile([P, 2 * TLOC], BF16, name="ee", tag="ee", bufs=6)
            nc.scalar.activation(ee[:], s_ps[:], AF.Exp, scale=0.125)
            for u in range(2):
                db = 2 * dp + u
                mcol = (8 + db) * CHUNK
                emr = pE.tile([P, CHUNK], BF16, name="emr", tag="emr", bufs=2)
                eng = nc.gpsimd if db % 2 == 0 else nc.vector
                eng.tensor_tensor(emr[:], ee[:, u * TLOC + CHUNK:
                                              u * TLOC + 2 * CHUNK],
                                  kmask_sb[:, mcol:mcol + CHUNK], ALU.mult)
                for qb in range(2):
                    av(yq, qb, db, ee[:, u * TLOC + qb * P:
                                      u * TLOC + (qb + 1) * P], h,
                       db == 0 and qb == 0, False)
                for qb in range(2, 4):
                    av(yq, qb, db, emr[:, (qb - 2) * P:(qb - 1) * P], h,
                       False, False)
        for g in range(2):
            s_ps = pps.tile([P, 2 * TLOC], F32, name="sps", tag="sps", bufs=2)
            for u in range(4):
                kb = 8 + 4 * g + u
                nc.tensor.matmul(s_ps[:, u * CHUNK:(u + 1) * CHUNK],
                                 kfull[mb][po:po + 64, kb * P:(kb + 1) * P],
                                 qbf[mb][po:po + 64, 0:CHUNK])
            ee2 = pE.tile([P, 2 * TLOC], BF16, name="ee2", tag="ee2", bufs=2)
            nc.scalar.activation(ee2[:], s_ps[:], AF.Exp, scale=0.125)
            mcol = 4 * g * CHUNK
            em2 = pE.tile([P, 2 * TLOC], BF16, name="em2", tag="em2", bufs=2)
            eng = nc.gpsimd if g % 2 == 0 else nc.vector
            eng.tensor_tensor(em2[:], ee2[:],
                              kmask_sb[:, mcol:mcol + 2 * TLOC], ALU.mult)
            for u in range(4):
                kb = 8 + 4 * g + u
                for qb in range(2):
                    av(yq, qb, kb,
                       em2[:, u * CHUNK + qb * P:u * CHUNK + (qb + 1) * P],
                       h, False, kb == 15 and qb == 1)
        yb = h // 4
        rr4 = ptmp2.tile([P, 4], F32, name="rr4", tag="rr")
        yqv = yq[:].rearrange("p (qb d) -> p qb d", qb=4)
        nc.vector.reciprocal(rr4[:], yqv[:, :, NOPE])
        for qb in range(4):
            base = qb * NO1
            key = (yb, qb)
            if key not in yt4:
                yt4[key] = pq2.tile([P, P], BF16, name=f"yt4_{yb}_{qb}")
            nc.vector.tensor_scalar(
                yt4[key][:, (h % 4) * NOPE:(h % 4 + 1) * NOPE],
                yq[:, base:base + NOPE], rr4[:, qb:qb + 1], None, op0=ALU.mult)
        if h % 4 == 3:
            for qb in range(4):
                tp = pps.tile([P, P], BF16, name="ytp", tag="qsm", bufs=2)
                nc.tensor.transpose(tp[:], yt4[(yb, qb)][:], ident_bf[:])
                nc.vector.tensor_scalar(
                    yall[yb // 2][:, yb % 2, qb * P:(qb + 1) * P],
                    tp[:], 0.0, None, op0=ALU.add)

    # ---- Q projection + rope, interleaved with per-head attention
    # (1-mb lookahead: Q for mb+1 is emitted before heads of mb)
    qbf = []

    def q_proj(mb):
        tl = pq1.tile([P, TLOC], BF16, name=f"qbf{mb}")
        for ch in range(2):
            csl = slice(ch * CHUNK, (ch + 1) * CHUNK)
            ps = pps.tile([P, CHUNK], F32, name="qps", tag="qsm", bufs=2)
            for dcb in range(4):
                nc.tensor.matmul(ps[:], wq8_sb[:, dcb, :, mb * P:(mb + 1) * P],
                                 xl8_t[:, 2 * dcb:2 * dcb + 2, csl],
                                 start=(dcb == 0), stop=(dcb == 3),
                                 perf_mode=DR)
            nc.vector.tensor_tensor(tl[:, csl], ps[:], bc1l[:, csl], ALU.mult)
            qr = ptmp1.tile([2 * ROPE, CHUNK], BF16, name="qr", tag="qr")
            nc.vector.tensor_tensor(qr[0:ROPE, :], ps[32:64, :],
                                    bc1l[32:64, csl], ALU.mult)
            nc.vector.tensor_tensor(qr[ROPE:2 * ROPE, :], ps[96:128, :],
                                    bc1l[96:128, csl], ALU.mult)
            par = pps.tile([2 * ROPE, CHUNK], F32, name="qpar", tag="qsm", bufs=2)
            nc.tensor.matmul(par[:], perm64_sb[:], qr[:])
            t1 = ptmp1.tile([2 * ROPE, CHUNK], BF16, name="qt1", tag="qt1")
            nc.gpsimd.tensor_tensor(t1[:], qr[:], cosq_sb[:, csl], ALU.mult)
            t2 = ptmp1.tile([2 * ROPE, CHUNK], BF16, name="qt2", tag="qt2")
            nc.vector.tensor_tensor(t2[:], par[:], ssinq_sb[:, csl], ALU.mult)
            nc.gpsimd.tensor_tensor(tl[32:64, csl], t1[0:ROPE, :],
                                    t2[0:ROPE, :], ALU.add)
            nc.vector.tensor_tensor(tl[96:128, csl], t1[ROPE:2 * ROPE, :],
                                    t2[ROPE:2 * ROPE, :], ALU.add)
        qbf.append(tl)

    q_proj(0)
    emit_vext(range(0, 8))
    q_proj(1)
    emit_vext(range(8, NTB))
    for mb in range(8):
        if mb + 2 < 8:
            q_proj(mb + 2)
        attn_head(2 * mb)
        attn_head(2 * mb + 1)

    early.close()
    pat1 = whole.enter_context(tc.tile_pool(name="pat1", bufs=1, side="right"))
    pmx  = whole.enter_context(tc.tile_pool(name="pmx", bufs=1, side="right"))
    pwdw = whole.enter_context(tc.tile_pool(name="wdw", bufs=1, side="right"))
    wd_sb = []
    for idx in range(E + 1):
        tl = pwdw.tile([P, 2, 2, C], F8, name=f"wd8s{idx}")
        qeng[idx % 4].dma_start(tl[:], wd8.ap()[idx * P:(idx + 1) * P, :])
        wd_sb.append(tl)

    # ---- Wo + residual -> xa^T (kept in f32 to the end)
    xa = []
    for cb in range(NCB):
        xa.append(pat1.tile([P, TLOC], F32, name=f"xa{cb}"))
    sps2 = pacc.tile([1, TLOC], F32, name="ssq2", tag="accA")
    for cb in range(NCB):
        for ch in range(2):
            csl = slice(ch * CHUNK, (ch + 1) * CHUNK)
            ps = pps.tile([P, CHUNK], F32, name="ops", tag="qsm", bufs=2)
            for dib in range(2):
                nc.tensor.matmul(ps[:], wo8_sb[:, dib, :, cb * P:(cb + 1) * P],
                                 yall[dib][:, :, csl],
                                 start=(dib == 0), stop=(dib == 1),
                                 perf_mode=DR)
            nc.vector.scalar_tensor_tensor(xa[cb][:, csl], ps[:], 1.0 / WSCALE,
                                           xloc[cb][:, csl],
                                           op0=ALU.mult, op1=ALU.add)
        xq = ptmp2.tile([P, TLOC], BF16, name="xsq2", tag="xsqB")
        nc.scalar.activation(xq[:], xa[cb][:], AF.Square)
        nc.tensor.matmul(sps2[:], ones_128x1[:], xq[:],
                         start=(cb == 0), stop=(cb == NCB - 1))
    if DEBUG:
        for cb in range(NCB):
            nc.sync.dma_start(dbg["d_xaT"].ap()[cb * P:(cb + 1) * P, :],
                              xa[cb][:])

    # ---- rmsnorm2 + xmoe (MoE-phase pool pmx)
    invr2 = pmx.tile([1, TLOC], F32)
    rr2 = ptmp2.tile([1, TLOC], F32, name="rms2", tag="rmsB")
    nc.scalar.activation(rr2[:], sps2[:], AF.Sqrt, bias=eps_sb[:], scale=1.0 / C)
    nc.vector.reciprocal(invr2[:], rr2[:])
    if DEBUG:
        nc.sync.dma_start(dbg["d_invr2"].ap(), invr2[:])
    bc2 = pmx.tile([P, TLOC], F32)
    nc.gpsimd.partition_broadcast(bc2[:], invr2[:])
    # normalized MoE input, fp8, DoubleRow pair layout: [:, j, :] = chan
    # block 2*dcb+j
    xmoe8 = []
    for dcb in range(4):
        tl = pmx.tile([P, 2, TLOC], F8, name=f"xmoe8_{dcb}")
        nc.gpsimd.tensor_tensor(tl[:, 0, :], xa[2 * dcb][:], bc2[:], ALU.mult)
        nc.vector.tensor_tensor(tl[:, 1, :], xa[2 * dcb + 1][:], bc2[:],
                                ALU.mult)
        xmoe8.append(tl)

    # ---- gate (fp32)
    bcomb = [pmx.tile([P, TLOC], BF16, name=f"bcomb{e}") for e in range(E)]
    for tb in range(4):
        tsl = slice(tb * P, (tb + 1) * P)
        g_ps = pps.tile([P, E], F32, name="gps", tag="sps", bufs=2)
        for cb in range(NCB):
            nc.tensor.matmul(g_ps[:], xa[cb][:, tsl], wgate_sb[:, cb, :],
                             start=(cb == 0), stop=(cb == NCB - 1))
        ir_ps = pps.tile([P, 1], F32, name="irps", tag="sps", bufs=2)
        nc.tensor.transpose(ir_ps[:], invr2[:, tsl], ones1f[:])
        ir_col = ptmp2.tile([P, 1], F32, name="ircol", tag="ircol")
        nc.scalar.copy(ir_col[:], ir_ps[:])
        lg = ptmp2.tile([P, E], F32, name="lg", tag="lg")
        nc.vector.scalar_tensor_tensor(lg[:], g_ps[:], ir_col[:], biasg_sb[:],
                                       op0=ALU.mult, op1=ALU.add)
        m1 = ptmp2.tile([P, 1], F32, name="m1", tag="m1")
        nc.vector.reduce_max(m1[:], lg[:], axis=mybir.AxisListType.X)
        eq1 = ptmp2.tile([P, E], F32, name="eq1", tag="eq1")
        nc.vector.tensor_scalar(eq1[:], lg[:], m1[:], None, op0=ALU.is_equal)
        lm = ptmp2.tile([P, E], F32, name="lm", tag="lm")
        nc.vector.scalar_tensor_tensor(lm[:], eq1[:], -1e9, lg[:],
                                       op0=ALU.mult, op1=ALU.add)
        m2 = ptmp2.tile([P, 1], F32, name="m2", tag="m2")
        nc.vector.reduce_max(m2[:], lm[:], axis=mybir.AxisListType.X)
        eq2 = ptmp2.tile([P, E], F32, name="eq2", tag="eq2")
        nc.vector.tensor_scalar(eq2[:], lm[:], m2[:], None, op0=ALU.is_equal)
        dm = ptmp2.tile([P, 1], F32, name="dm", tag="dm")
        nc.vector.tensor_scalar(dm[:], m1[:], m2[:], None, op0=ALU.subtract)
        w1 = ptmp2.tile([P, 1], F32, name="w1", tag="w1")
        nc.scalar.activation(w1[:], dm[:], AF.Sigmoid)
        w2 = ptmp2.tile([P, 1], F32, name="w2", tag="w2")
        nc.vector.tensor_scalar(w2[:], w1[:], -1.0, 1.0, op0=ALU.mult,
                                op1=ALU.add)
        cmb = ptmp2.tile([P, E], F32, name="cmb", tag="cmb")
        nc.vector.tensor_scalar(cmb[:], eq1[:], w1[:], HSCALE, op0=ALU.mult,
                                op1=ALU.mult)
        cm2 = ptmp2.tile([P, E], F32, name="cm2", tag="cm2")
        nc.vector.tensor_scalar(cm2[:], eq2[:], w2[:], HSCALE, op0=ALU.mult,
                                op1=ALU.mult)
        cmf = ptmp2.tile([P, E], F32, name="cmf", tag="cmf")
        nc.vector.tensor_tensor(cmf[:], cmb[:], cm2[:], ALU.add)
        if DEBUG:
            nc.sync.dma_start(dbg["d_comb"].ap()[:, tb * E:(tb + 1) * E],
                              cmf[:])
        ct_ps = pps.tile([E, P], F32, name="ctps", tag="sps", bufs=2)
        nc.tensor.transpose(ct_ps[:], cmf[:], ident_sb[:])
        ct_sb = ptmp2.tile([E, P], BF16, name="ctsb", tag="ctsb")
        nc.scalar.copy(ct_sb[:], ct_ps[:])
        for e in range(E):
            bc_ps = pps.tile([P, P], F32, name="bcps", tag="qsm", bufs=2)
            nc.tensor.matmul(bc_ps[:], sel8_sb[:, e * P:(e + 1) * P],
                             ct_sb[:])
            if e % 2 == 0:
                nc.scalar.copy(bcomb[e][:, tsl], bc_ps[:])
            else:
                nc.vector.tensor_scalar(bcomb[e][:, tsl], bc_ps[:], 0.0,
                                        None, op0=ALU.add)

    attn.close()

    # ---- MoE: fp8 DoubleRow matmuls, PSUM accumulation across all experts.
    # Expert 0 = shared (gating = HSCALE), experts 1..8 gated by bcomb
    # (already scaled by HSCALE).  All weights are pre-scaled by WSCALE;
    # compensations: silu scale=1/WSCALE, hh mult 1/WSCALE, final add
    # 1/(WSCALE*HSCALE).
    moe = contextlib.ExitStack()
    pw   = moe.enter_context(tc.tile_pool(name="wmoe", bufs=1))
    pgu  = moe.enter_context(tc.tile_pool(name="psG", bufs=3, space="PSUM"))
    pwd  = moe.enter_context(tc.tile_pool(name="psD", bufs=1, space="PSUM"))
    pmoe = moe.enter_context(tc.tile_pool(name="hmoe", bufs=3))
    ph8  = moe.enter_context(tc.tile_pool(name="h8p", bufs=1))

    NEXP = E + 1
    NCB_W = 5               # wd psum banks held through phase A
    h8 = [[ph8.tile([P, 2, TLOC], F8, name=f"h8_{idx}_{dib}")
           for dib in range(2)] for idx in range(NEXP)]
    wdps = [pwd.tile([P, TLOC], F32, name=f"wdps{cb}")
            for cb in range(NCB_W)]

    def wd_partial(idx, cbs, tiles):
        for cb, wt in zip(cbs, tiles):
            csl = slice(cb * P, (cb + 1) * P)
            for dib in range(2):
                nc.tensor.matmul(wt[:], wd_sb[idx][:, dib, :, csl],
                                 h8[idx][dib][:],
                                 start=(idx == 0 and dib == 0),
                                 stop=(idx == NEXP - 1 and dib == 1),
                                 perf_mode=DR)

    def expert_gu(idx):
        wg_sb = pw.tile([P, 4, 2, I], F8, name="wg8s", tag=f"wg{idx % 2}")
        nc.sync.dma_start(wg_sb[:], wg8.ap()[idx * P:(idx + 1) * P, :])
        wu_sb = pw.tile([P, 4, 2, I], F8, name="wu8s", tag=f"wu{idx % 2}")
        nc.sync.dma_start(wu_sb[:], wu8.ap()[idx * P:(idx + 1) * P, :])
        for ib in range(NIB):
            isl = slice(ib * P, (ib + 1) * P)
            gp = pgu.tile([P, TLOC], F32, name="gp", tag="psG")
            for dcb in range(4):
                nc.tensor.matmul(gp[:], wg_sb[:, dcb, :, isl], xmoe8[dcb][:],
                                 start=(dcb == 0), stop=(dcb == 3),
                                 perf_mode=DR)
            sg = pmoe.tile([P, TLOC], BF16, name="sg", tag="sg")
            nc.scalar.activation(sg[:], gp[:], AF.Silu, scale=1.0 / WSCALE)
            up = pgu.tile([P, TLOC], F32, name="up", tag="psG")
            for dcb in range(4):
                nc.tensor.matmul(up[:], wu_sb[:, dcb, :, isl], xmoe8[dcb][:],
                                 start=(dcb == 0), stop=(dcb == 3),
                                 perf_mode=DR)
            hh = pmoe.tile([P, TLOC], BF16, name="hh", tag="hh")
            nc.vector.scalar_tensor_tensor(hh[:], up[:], 1.0 / WSCALE, sg[:],
                                           op0=ALU.mult, op1=ALU.mult)
            dst = h8[idx][ib // 2][:, ib % 2, :]
            if idx == 0:
                nc.gpsimd.tensor_scalar(dst, hh[:], HSCALE, None, op0=ALU.mult)
            else:
                nc.gpsimd.tensor_tensor(dst, hh[:], bcomb[idx - 1][:],
                                        ALU.mult)

    # phase A with 1-expert-lagged partial Wd for cb < NCB_W
    for idx in range(NEXP):
        expert_gu(idx)
        if idx > 1:
            wd_partial(idx - 2, range(NCB_W), wdps)
    wd_partial(NEXP - 2, range(NCB_W), wdps)
    wd_partial(NEXP - 1, range(NCB_W), wdps)
    for cb in range(NCB_W):
        fo = pmoe.tile([P, TLOC], F32, name="fo", tag="fo")
        nc.vector.scalar_tensor_tensor(fo[:], wdps[cb][:],
                                       1.0 / (WSCALE * HSCALE),
                                       xa[cb][:], op0=ALU.mult, op1=ALU.add)
        nc.sync.dma_start(outT.ap()[cb * P:(cb + 1) * P, :], fo[:])
    # tail: remaining cb reuse the freed g/u psum slots
    for cb in range(NCB_W, NCB):
        wt = pgu.tile([P, TLOC], F32, name="wdtail", tag="psG")
        for idx in range(NEXP):
            csl = slice(cb * P, (cb + 1) * P)
            for dib in range(2):
                nc.tensor.matmul(wt[:], wd_sb[idx][:, dib, :, csl],
                                 h8[idx][dib][:],
                                 start=(idx == 0 and dib == 0),
                                 stop=(idx == NEXP - 1 and dib == 1),
                                 perf_mode=DR)
        fo = pmoe.tile([P, TLOC], F32, name="fo", tag="fo")
        nc.vector.scalar_tensor_tensor(fo[:], wt[:],
                                       1.0 / (WSCALE * HSCALE),
                                       xa[cb][:], op0=ALU.mult, op1=ALU.add)
        nc.sync.dma_start(outT.ap()[cb * P:(cb + 1) * P, :], fo[:])

    moe.close()
    whole.close()


# =============================================================== host side
def _build():
    if "nc" in _CACHE:
        return _CACHE["nc"]
    nc = bacc.Bacc("TRN2", target_bir_lowering=False, debug=False,
                   num_devices=8)
    with tile.TileContext(nc) as tc:
        _emit(nc, tc)
    nc.compile()
    _CACHE["nc"] = nc
    return nc


def _rope_tables(pos):
    # pos: (N,) positions; returns cos,ssin of shape (ROPE, N) in the
    # row-pair layout (rows 2i/2i+1 both carry angle pos*freq_i; ssin row 2i
    # is -sin, row 2i+1 is +sin).
    freqs = 1.0 / (THETA ** (np.arange(0, ROPE, 2, dtype=np.float32) / ROPE))
    ang = np.outer(freqs, pos.astype(np.float32))          # (16, N)
    cos = np.repeat(np.cos(ang), 2, axis=0).astype(np.float32)
    sin = np.sin(ang).astype(np.float32)
    ssin = np.empty((ROPE, len(pos)), np.float32)
    ssin[0::2] = -sin
    ssin[1::2] = sin
    return cos, ssin


def _host_inputs(inputs, core):
    bf = lambda a: np.ascontiguousarray(a).astype(ml_dtypes.bfloat16)
    f32 = lambda a: np.ascontiguousarray(a, dtype=np.float32)
    b, q = core // 4, core % 4
    hi, lo = 7 - q, q           # slot0 = chunk hi, slot1 = chunk lo
    x = np.asarray(inputs["x"], np.float32)
    w_ln1 = np.asarray(inputs["w_ln1"], np.float32)
    w_ln2 = np.asarray(inputs["w_ln2"], np.float32)
    xT = x[b].T                                            # (C, T)
    loc_cols = np.r_[np.arange(hi * CHUNK, (hi + 1) * CHUNK),
                     np.arange(lo * CHUNK, (lo + 1) * CHUNK)]
    xloc = xT[:, loc_cols]

    # rope tables
    posq = loc_cols.astype(np.float32)
    cq, sq = _rope_tables(posq)
    cosq = np.vstack([cq, cq])                             # (64, 512)
    ssinq = np.vstack([sq, sq])
    posk = np.arange(T, dtype=np.float32)
    coskp, ssinkp = _rope_tables(posk)           # (32, 2048) = [32, nt*512+t]

    # permutation matrices (pair swap)
    p32 = np.zeros((ROPE, ROPE), np.float32)
    for i in range(ROPE // 2):
        p32[2 * i + 1, 2 * i] = 1.0
        p32[2 * i, 2 * i + 1] = 1.0
    p64 = np.zeros((2 * ROPE, 2 * ROPE), np.float32)
    p64[:ROPE, :ROPE] = p32
    p64[ROPE:, ROPE:] = p32

    # causal masks per (slot, kb)
    kmask = np.zeros((P, NKB * CHUNK), np.float32)
    ki = np.arange(P)[:, None]
    qi = np.arange(CHUNK)[None, :]
    for sl, j in ((0, hi), (1, lo)):
        base = sl * KB_SLOT[0] * CHUNK
        for kb in range(KB_SLOT[sl]):
            m = np.zeros((P, CHUNK), np.float32)
            if kb < 2 * j:
                m[:] = 1.0
            elif kb == 2 * j:
                m = (ki <= qi).astype(np.float32)
            elif kb == 2 * j + 1:
                m = (ki + P <= qi).astype(np.float32)
            kmask[:, base + kb * CHUNK: base + (kb + 1) * CHUNK] = m

    wq = np.asarray(inputs["Wq"], np.float32) * w_ln1[:, None]
    wkva = np.asarray(inputs["Wkva"], np.float32) * w_ln1[:, None]
    wo_nope = np.asarray(inputs["Wo"], np.float32).reshape(H, D, C)[:, :NOPE]
    wgate = np.asarray(inputs["Wgate"], np.float32) * w_ln2[:, None]
    biasg = np.broadcast_to(np.asarray(inputs["expert_bias"], np.float32),
                            (P, E)).copy()

    # fp8 expert weights with expert 0 = shared; DoubleRow pair packing.
    fp8 = ml_dtypes.float8_e4m3
    wg_all = np.concatenate(
        [np.asarray(inputs["sWg"], np.float32)[0:1],
         np.asarray(inputs["Wg"], np.float32)], axis=0) * w_ln2[None, :, None]
    wu_all = np.concatenate(
        [np.asarray(inputs["sWu"], np.float32)[0:1],
         np.asarray(inputs["Wu"], np.float32)], axis=0) * w_ln2[None, :, None]
    wd_all = np.concatenate(
        [np.asarray(inputs["sWd"], np.float32)[0:1],
         np.asarray(inputs["Wd"], np.float32)], axis=0)

    def pack_pairs(w, nblk):
        # w: (NEXP, K, M) with K = 256*nblk -> (NEXP*128, nblk*2*M):
        # row e*128+k, col [blk, j, m] = w[e, 256*blk + 128*j + k, m]
        ne, kk, mm = w.shape
        assert kk == 256 * nblk
        r = w.reshape(ne, nblk, 2, P, mm).transpose(0, 3, 1, 2, 4)
        return np.ascontiguousarray(r.reshape(ne * P, nblk * 2 * mm))

    fp8c = lambda a: np.clip(a, -240, 240).astype(ml_dtypes.float8_e4m3)
    m = {
        "xT8": fp8c(xT),
        "xlocT": f32(xloc),
        "xloc8": fp8c(xloc),
        "wq8": pack_pairs(wq[None] * WSCALE, 4).astype(ml_dtypes.float8_e4m3),
        "wkva8": pack_pairs(wkva[None] * WSCALE, 4).astype(ml_dtypes.float8_e4m3),
        "wkvb": bf(inputs["Wkvb"]),
        "wo8": pack_pairs(wo_nope.reshape(H * NOPE, C)[None]
                          * WSCALE, 2).astype(fp8),
        "cosq": bf(cosq), "ssinq": bf(ssinq),
        "coskp": bf(coskp), "ssinkp": bf(ssinkp),
        "perm64": bf(p64), "perm32": bf(p32),
        "ident": np.eye(P, dtype=np.float32),
        "identbf": np.eye(P, dtype=ml_dtypes.bfloat16),
        "sel8": np.kron(np.eye(E, dtype=np.float32),
                        np.ones((1, P), np.float32)).astype(ml_dtypes.bfloat16),
        "kmask": bf(kmask),
        "wgate": f32(wgate),
        "biasg": biasg,
        "wg8": pack_pairs(wg_all * WSCALE, 4).astype(fp8),
        "wu8": pack_pairs(wu_all * WSCALE, 4).astype(fp8),
        "wd8": pack_pairs(wd_all * WSCALE, 2).astype(fp8),
    }
    return m


def kernel(**inputs):
    nc = _build()
    in_maps = [_host_inputs(inputs, core) for core in range(8)]
    kw = {}
    if os.environ.get("BASSK_TRACE"):
        kw = dict(trace=True, tmpdir=os.environ.get("BASSK_TRACE_DIR") or None)
    res = bass_utils.run_bass_kernel_spmd(nc, in_maps, core_ids=list(range(8)),
                                          **kw)
    if os.environ.get("BASSK_TRACE"):
        print(f"PROFILE exec_time_ns: {res.exec_time_ns}")
        print(f"PROFILE json: {res.profile_json}")
    out = np.empty((B, T, C), np.float32)
    for core in range(8):
        b, q = core // 4, core % 4
        hi, lo = 7 - q, q
        oT = res.results[core]["outT"]                     # (C, 512)
        out[b, hi * CHUNK:(hi + 1) * CHUNK] = oT[:, :CHUNK].T
        out[b, lo * CHUNK:(lo + 1) * CHUNK] = oT[:, CHUNK:].T
    return out



# revision 80
# speedup vs baseline: 1.0451x; 1.0451x over previous
# DeepSeek block (MLA attention + top-2-of-8 MoE + shared expert) on 8 TRN2
# NeuronCores, zero-collective sharding.
#
# Core c in [0..8): sequence b = c//4, q = c%4; owns token chunks
# hi = 7-q (slot 0) and lo = q (slot 1), 256 tokens each (causally balanced:
# every core's true causal work is 9 key blocks of 256).
#
# The SPMD program is identical on all cores; everything core-specific
# arrives as data: slot 0 attends key blocks [0..16), slot 1 [0..8), with
# host-built multiplicative masks (ones for fully-past blocks, triangular at
# the causal boundary, zeros for fully-future blocks).
#
# Layout: activations live feature-on-partition ("^T", tokens on the free
# axis). Host pre-transposes x, folds w_ln1/w_ln2 into adjacent weights,
# pre-casts weights to bf16, and builds rope tables / masks / identity.
# Matmuls run in bf16 (fp32 PSUM accumulate); the gate runs in fp32.
import os
import numpy as np
import ml_dtypes

import concourse.bacc as bacc
import concourse.mybir as mybir
import concourse.tile as tile
from concourse import bass_utils

F32 = mybir.dt.float32
F32R = mybir.dt.float32r
BF16 = mybir.dt.bfloat16
F8 = mybir.dt.float8e4
DR = mybir.MatmulPerfMode.DoubleRow
AF = mybir.ActivationFunctionType
ALU = mybir.AluOpType

WSCALE = 64.0            # fp8 expert-weight scale (keeps values normal-range)
HSCALE = 16.0            # fp8 h-activation scale (folded into comb weights)

B, T, C, H, D = 2, 2048, 1024, 16, 64
R, ROPE, NOPE = 128, 32, 32
E, I = 8, 512
THETA, EPS = 100000.0, 1e-5
P = 128
NCB = C // P             # 8 C blocks
NTB = T // P             # 16 key/token blocks per sequence
TLOC, CHUNK = 512, 256
KB_SLOT = (16, 8)        # key blocks attended per chunk slot
NKB = sum(KB_SLOT)
NIB = I // P             # 4 I blocks

DEBUG = bool(int(os.environ.get("BASSK_DEBUG", "0")))
_CACHE = {}


# =============================================================== device IR
def _emit(nc, tc):
    import contextlib

    def din(name, shape, dt):
        return nc.dram_tensor(name, shape, dt, kind="ExternalInput")

    xT8    = din("xT8", (C, T), F8)
    xlocT  = din("xlocT", (C, TLOC), F32)
    xloc8  = din("xloc8", (C, TLOC), F8)
    wq8    = din("wq8", (P, 4 * 2 * H * D), F8)    # DoubleRow pairs, x WSCALE
    wkva8  = din("wkva8", (P, 4 * 2 * (R + ROPE)), F8)
    wkvb   = din("wkvb", (R, H * NOPE), BF16)
    wo8    = din("wo8", (P, 2 * 2 * C), F8)   # DR pairs, x WSCALE
    cosq   = din("cosq", (2 * ROPE, TLOC), BF16)
    ssinq  = din("ssinq", (2 * ROPE, TLOC), BF16)
    coskp  = din("coskp", (ROPE, 4 * 512), BF16)  # [32, nt*512+t]
    ssinkp = din("ssinkp", (ROPE, 4 * 512), BF16)
    perm64 = din("perm64", (2 * ROPE, 2 * ROPE), BF16)
    perm32 = din("perm32", (ROPE, ROPE), BF16)
    ident  = din("ident", (P, P), F32)
    identbf = din("identbf", (P, P), BF16)
    sel8   = din("sel8", (E, E * P), BF16)   # sel8[k, e*128+m] = (k == e)
    kmask  = din("kmask", (P, NTB * P), BF16)
    wgate  = din("wgate", (C, E), F32)
    biasg  = din("biasg", (P, E), F32)
    # fp8 expert weights, expert 0 = shared.  wg8/wu8 rows e*P+k hold
    # [dcb, j, m] -> W[256*dcb + 128*j + k, m] * WSCALE (DoubleRow pairs on j).
    # wd8 rows e*P+k hold [dib, j, c] -> Wd[256*dib + 128*j + k, c] * WSCALE.
    wg8    = din("wg8", ((E + 1) * P, 4 * 2 * I), F8)
    wu8    = din("wu8", ((E + 1) * P, 4 * 2 * I), F8)
    wd8    = din("wd8", ((E + 1) * P, 2 * 2 * C), F8)

    outT = nc.dram_tensor("outT", (C, TLOC), F32, kind="ExternalOutput")
    dbg = {}
    if DEBUG:
        for name, shape in [("d_xaT", (C, TLOC)), ("d_comb", (P, 4 * E)),
                            ("d_invr1", (1, T)), ("d_invr2", (1, TLOC))]:
            dbg[name] = nc.dram_tensor(name, shape, F32, kind="ExternalOutput")

    f32r = lambda ap: ap.bitcast(F32R)

    # ---------------- pools (sized to fit 192KB/partition SBUF, 8 PSUM banks)
    whole = contextlib.ExitStack()   # whole kernel
    attn  = contextlib.ExitStack()   # until gate done
    early = contextlib.ExitStack()   # until Q/ckv/krope done
    xtst  = contextlib.ExitStack()   # xt tiles, until ckv done
    pc   = whole.enter_context(tc.tile_pool(name="pc", bufs=1))
    pq1  = attn.enter_context(tc.tile_pool(name="pq1", bufs=1))
    pkv0 = attn.enter_context(tc.tile_pool(name="pkv0", bufs=1))
    pps  = attn.enter_context(tc.tile_pool(name="psA", bufs=2, space="PSUM"))
    pacc = attn.enter_context(tc.tile_pool(name="psB", bufs=2, space="PSUM"))
    px   = early.enter_context(tc.tile_pool(name="px", bufs=1, side="right"))
    ptmp1 = early.enter_context(tc.tile_pool(name="tmp1", bufs=2, side="right"))
    pxt  = xtst.enter_context(tc.tile_pool(name="pxt", bufs=1, side="right"))

    # ---- constants
    ones_128x1 = pc.tile([P, 1], BF16); nc.any.memset(ones_128x1[:], 1.0)
    ones1f = pc.tile([1, 1], F32); nc.any.memset(ones1f[:], 1.0)
    eps_sb = pc.tile([1, 1], F32); nc.any.memset(eps_sb[:], EPS)
    eps64 = pc.tile([1, 1], F32)
    nc.any.memset(eps64[:], EPS * WSCALE * WSCALE)
    ident_sb = pc.tile([P, P], F32); nc.sync.dma_start(ident_sb[:], ident.ap())
    ident_bf = pc.tile([P, P], BF16)
    nc.sync.dma_start(ident_bf[:], identbf.ap())
    sel8_sb = pc.tile([E, E * P], BF16)
    nc.sync.dma_start(sel8_sb[:], sel8.ap())
    perm64_sb = pc.tile([2 * ROPE, 2 * ROPE], BF16)
    nc.sync.dma_start(perm64_sb[:], perm64.ap())
    perm32_sb = pc.tile([ROPE, ROPE], BF16)
    nc.sync.dma_start(perm32_sb[:], perm32.ap())
    biasg_sb = pc.tile([P, E], F32); nc.sync.dma_start(biasg_sb[:], biasg.ap())
    cosq_sb = px.tile([2 * ROPE, TLOC], BF16)
    nc.sync.dma_start(cosq_sb[:], cosq.ap())
    ssinq_sb = px.tile([2 * ROPE, TLOC], BF16)
    nc.sync.dma_start(ssinq_sb[:], ssinq.ap())
    coskp_all = px.tile([ROPE, 4, 512], BF16, name="coskp_all")
    nc.sync.dma_start(coskp_all[:], coskp.ap())
    coskp_sb = [coskp_all[:, nt, :] for nt in range(4)]
    ssinkp_all = px.tile([ROPE, 4, 512], BF16, name="ssinkp_all")
    nc.gpsimd.dma_start(ssinkp_all[:], ssinkp.ap())
    ssinkp_sb = [ssinkp_all[:, nt, :] for nt in range(4)]

    # ---- bulk loads (early), DMA issues spread across engine queues
    qeng = [nc.sync, nc.scalar, nc.sync, nc.gpsimd, nc.sync, nc.scalar]
    def load_blocks(pool, name, dram_ap, nblk, width, dt, nsplit, qoff=0):
        # one [P, nblk, width] tile, DMA'd in nsplit chunks across queues
        tl = pool.tile([P, nblk, width], dt, name=name)
        per = nblk // nsplit
        src_v = dram_ap.rearrange("(cb p) f -> p cb f", p=P)
        for s in range(nsplit):
            qeng[(qoff + s) % 6].dma_start(
                tl[:, s * per:(s + 1) * per, :],
                src_v[:, s * per:(s + 1) * per, :])
        return [tl[:, cb, :] for cb in range(nblk)]

    xt_t = pxt.tile([P, NCB, T], F8, name="xt8_all")
    for s in range(NCB):
        qeng[s % 6].dma_start(
            xt_t[:, s:s + 1, :],
            xT8.ap().rearrange("(cb p) f -> p cb f", p=P)[:, s:s + 1, :])
    xt = [xt_t[:, cb, :] for cb in range(NCB)]
    xloc = load_blocks(pq1, "xloc_all", xlocT.ap(), NCB, TLOC, F32, 2, 4)
    xl8_t = px.tile([P, NCB, TLOC], F8, name="xl8_all")
    nc.scalar.dma_start(xl8_t[:],
                        xloc8.ap().rearrange("(cb p) f -> p cb f", p=P))
    wq8_sb = px.tile([P, 4, 2, H * D], F8, name="wq8_sb")
    for s in range(2):
        qeng[s].dma_start(wq8_sb[:, 2 * s:2 * s + 2, :, :],
                          wq8.ap().rearrange("p (d j f) -> p d j f", d=4, j=2)
                          [:, 2 * s:2 * s + 2, :, :])
    wkva8_sb = px.tile([P, 4, 2, R + ROPE], F8, name="wkva8_sb")
    nc.gpsimd.dma_start(wkva8_sb[:],
                        wkva8.ap().rearrange("p (d j f) -> p d j f", d=4, j=2))
    wkvb_sb = pkv0.tile([R, H * NOPE], BF16, name="wkvbs")
    nc.gpsimd.dma_start(wkvb_sb[:], wkvb.ap())

    # ---- rmsnorm1 stats: global (keys) then local (queries)
    invr1 = px.tile([1, T], BF16)
    bc1 = px.tile([P, T], BF16)
    for nt in range(T // 512):
        sl = slice(nt * 512, (nt + 1) * 512)
        sps = pacc.tile([1, 512], F32, name="ssq1", tag="accA")
        for cb in range(NCB):
            xq = ptmp1.tile([P, 512], BF16, name="xsq", tag="xsq")
            if cb % 2 == 0:
                nc.scalar.activation(xq[:], xt[cb][:, sl], AF.Square)
            else:
                nc.vector.tensor_tensor(xq[:], xt[cb][:, sl], xt[cb][:, sl],
                                        ALU.mult)
            nc.tensor.matmul(sps[:], ones_128x1[:], xq[:],
                             start=(cb == 0), stop=(cb == NCB - 1))
        rr = ptmp1.tile([1, 512], F32, name="rms1", tag="rms1")
        nc.scalar.activation(rr[:], sps[:], AF.Sqrt, bias=eps64[:],
                             scale=WSCALE * WSCALE / C)
        with nc.allow_low_precision(reason="rmsnorm scale in bf16"):
            nc.vector.reciprocal(invr1[:, sl], rr[:])
        nc.gpsimd.partition_broadcast(bc1[:, sl], invr1[:, sl])


    # ---- ckv (scaled);  k_rope raw kept packed in two (64,512) tiles
    kvlat = pkv0.tile([R, T], BF16)
    kropef = [px.tile([ROPE, 512], BF16, name=f"kropef{i}") for i in range(4)]
    for nt in range(T // 512):
        sl = slice(nt * 512, (nt + 1) * 512)
        lat_ps = pps.tile([P, 512], F32, name="latps", tag="sps", bufs=2)
        rop_ps = pps.tile([ROPE, 512], F32, name="ropps", tag="sps", bufs=2)
        for dcb in range(4):
            nc.tensor.matmul(lat_ps[:], wkva8_sb[:, dcb, :, 0:R],
                             xt_t[:, 2 * dcb:2 * dcb + 2, sl],
                             start=(dcb == 0), stop=(dcb == 3), perf_mode=DR)
        for dcb in range(4):
            nc.tensor.matmul(rop_ps[:], wkva8_sb[:, dcb, :, R:R + ROPE],
                             xt_t[:, 2 * dcb:2 * dcb + 2, sl],
                             start=(dcb == 0), stop=(dcb == 3), perf_mode=DR)
        nc.vector.tensor_tensor(kvlat[:, sl], lat_ps[:], bc1[:, sl], ALU.mult)
        nc.vector.tensor_tensor(kropef[nt][:], rop_ps[:],
                                bc1[0:ROPE, sl], ALU.mult)

    invr1l = px.tile([1, TLOC], F32)
    bc1l = px.tile([P, TLOC], F32)
    spsl = pacc.tile([1, TLOC], F32, name="ssql", tag="accA")
    for cb in range(NCB):
        xq = ptmp1.tile([P, TLOC], BF16, name="xsql", tag="xsq")
        nc.scalar.activation(xq[:], xloc[cb][:], AF.Square)
        nc.tensor.matmul(spsl[:], ones_128x1[:], xq[:],
                         start=(cb == 0), stop=(cb == NCB - 1))
    rrl = ptmp1.tile([1, TLOC], F32, name="rmsl", tag="rms1")
    nc.scalar.activation(rrl[:], spsl[:], AF.Sqrt, bias=eps64[:],
                         scale=WSCALE * WSCALE / C)
    nc.vector.reciprocal(invr1l[:], rrl[:])
    nc.gpsimd.partition_broadcast(bc1l[:], invr1l[:])

    # ---- rope K -> kr32 (32, T)
    kr32 = pkv0.tile([ROPE, T], BF16)
    for nt in range(T // 512):
        sl = slice(nt * 512, (nt + 1) * 512)
        src = kropef[nt][:]
        par_ps = pps.tile([ROPE, 512], F32, name="parps", tag="sps", bufs=2)
        nc.tensor.matmul(par_ps[:], perm32_sb[:], src)
        t1 = ptmp1.tile([ROPE, 512], BF16, name="kr1", tag="kr1")
        nc.vector.tensor_tensor(t1[:], src, coskp_sb[nt][:], ALU.mult)
        t2 = ptmp1.tile([ROPE, 512], BF16, name="kr2", tag="kr2")
        nc.vector.tensor_tensor(t2[:], par_ps[:], ssinkp_sb[nt][:], ALU.mult)
        nc.vector.tensor_tensor(kr32[:, sl], t1[:], t2[:], ALU.add)

    xtst.close()
    pkv  = attn.enter_context(tc.tile_pool(name="pkv", bufs=1))
    pq2  = attn.enter_context(tc.tile_pool(name="pq2", bufs=1))
    ptmp2 = attn.enter_context(tc.tile_pool(name="tmp2", bufs=2))
    pE   = attn.enter_context(tc.tile_pool(name="pE", bufs=4))

    # ---- k_nope (transient), V_ext, kfull (emitted before Q so the PE/Act/
    # DMA work here overlaps the DVE-heavy Q phase)
    sub = contextlib.ExitStack()
    pkx = sub.enter_context(tc.tile_pool(name="pkx", bufs=2))
    kfull = [pkv.tile([P, T], BF16, name=f"kfull{mb}") for mb in range(8)]
    for half in range(2):
        knope = []
        for mb in (2 * half, 2 * half + 1):
            tl = pkx.tile([P, T], BF16, name="knope", tag="kn")
            for nt in range(T // 512):
                sl = slice(nt * 512, (nt + 1) * 512)
                ps = pps.tile([P, 512], F32, name="knps", tag="sps", bufs=2)
                nc.tensor.matmul(ps[:], wkvb_sb[:, mb * P:(mb + 1) * P],
                                 kvlat[:, sl])
                if nt % 2 == 0:
                    nc.scalar.copy(tl[:, sl], ps[:])
                else:
                    nc.vector.tensor_scalar(tl[:, sl], ps[:], 0.0, None,
                                            op0=ALU.add)
            knope.append(tl)
        for mb in range(4 * half, 4 * half + 4):
            tl = kfull[mb]
            h0, h1 = 2 * mb, 2 * mb + 1
            e0, e1 = (nc.sync, nc.gpsimd) if mb % 2 else (nc.gpsimd, nc.sync)
            e0.dma_start(tl[0:32, :],
                         knope[h0 // 4 - 2 * half]
                         [(h0 % 4) * 32:(h0 % 4) * 32 + 32, :])
            e1.dma_start(tl[64:96, :],
                         knope[h1 // 4 - 2 * half]
                         [(h1 % 4) * 32:(h1 % 4) * 32 + 32, :])
            e0.dma_start(tl[32:64, :], kr32[:])
            e1.dma_start(tl[96:128, :], kr32[:])
    sub.close()
    vext = [pkv.tile([P, H, 34], BF16, name=f"vext{tb}")
            for tb in range(NTB)]

    def emit_vext(tbs):
        for tb in tbs:
            tl = vext[tb]
            ps = pps.tile([P, H * NOPE], F32, name="vps", tag="sps", bufs=2)
            nc.tensor.matmul(ps[:], kvlat[:, tb * P:(tb + 1) * P], wkvb_sb[:])
            if tb % 2 == 0:
                nc.scalar.copy(tl[:, :, 0:NOPE],
                               ps[:].rearrange("p (h d) -> p h d", h=H))
            else:
                nc.vector.tensor_scalar(
                    tl[:, :, 0:NOPE],
                    ps[:].rearrange("p (h d) -> p h d", h=H),
                    0.0, None, op0=ALU.add)
            nc.any.memset(tl[:, :, NOPE:NOPE + 1], 1.0)

    # deferred loads for the scores/Wo/MoE phases (DMAs overlap attention)
    kmask_sb = pkv.tile([P, NTB * P], BF16)
    nc.gpsimd.dma_start(kmask_sb[:], kmask.ap())
    wo8_sb = pq2.tile([P, 2, 2, C], F8, name="wo8_sb")
    nc.sync.dma_start(wo8_sb[:],
                      wo8.ap().rearrange("p (d j f) -> p d j f", d=2, j=2))
    wgate_sb = pq2.tile([P, NCB, E], F32, name="wgate_sb")
    nc.gpsimd.dma_start(wgate_sb[:], wgate.ap().rearrange(
        "(cb p) e -> p cb e", p=P))

    # ---- attention core defs (q-major AV; exp in merged 512-wide tiles)
    # slot0 (q cols 0:256) attends kb 0..16, slot1 (cols 256:512) kb 0..8.
    # kb<8: one (128,512) score tile covers both slots; slot0 half needs no
    # mask (always fully-past), slot1 half gets kmask.  kb>=8 (slot0 only):
    # pairs (8+2p, 9+2p) share a (128,512) tile, masked as one 512-wide mult.
    # AV accumulates q-major: yq[:, qb*33:qb*33+33] = sum_k em[k,q] vext[k,:].
    yall = [pq2.tile([P, 2, TLOC], F8, name=f"yall{d}") for d in range(2)]
    yt4 = {}
    NO1 = NOPE + 1

    def av(yq, qb, kb, lhsT, h, start, stop):
        nc.tensor.matmul(yq[:, qb * NO1:(qb + 1) * NO1], lhsT,
                         vext[kb][:, h, 0:NO1], start=start, stop=stop)

    # striped decomposition: 4 query slots of 128 tokens with causal needs
    # (16, 12, 8, 4) key-blocks (identical on every core; masks are data).
    # q cols = [slot0 | slot1 | slot2 | slot3].  At iteration kb the active
    # width is (4 - kb//4)*128 and exactly the LAST active slot is masked.
    STILES = [([0, 1], 512), ([2, 3], 512), ([4, 5], 384), ([6, 7], 384),
              ([8, 9, 10, 11], 256), ([12, 13, 14, 15], 128)]

    def attn_head(h):
        mb, po = h // 2, (h % 2) * 64
        yq = pacc.tile([P, 4 * NO1], F32, name="yq", tag="accA")
        for ti, (kbs, w) in enumerate(STILES):
            nk = len(kbs)
            nn = nk * w
            subw = TLOC if w == 384 else w   # pad 384 to the bank boundary
            s_ps = pps.tile([P, nk, subw], F32, name="sps", tag="sps", bufs=2)
            for i, kb in enumerate(kbs):
                nc.tensor.matmul(s_ps[:, i, 0:w],
                                 kfull[mb][po:po + 64, kb * P:(kb + 1) * P],
                                 qbf[mb][po:po + 64, 0:w])
            ee = pE.tile([P, nn], BF16, name="ee", tag="ee", bufs=6)
            nc.scalar.activation(ee[:].rearrange("p (i c) -> p i c", i=nk),
                                 s_ps[:, :, 0:w], AF.Exp, scale=0.125)
            emq = pE.tile([P, nk * P], BF16, name="emq", tag="emq", bufs=2)
            eng = nc.gpsimd if ti % 2 == 0 else nc.vector
            eng.tensor_tensor(
                emq[:].rearrange("p (i c) -> p i c", i=nk),
                ee[:].rearrange("p (i c) -> p i c", i=nk)[:, :, w - P:w],
                kmask_sb[:, kbs[0] * P:(kbs[-1] + 1) * P]
                .rearrange("p (i c) -> p i c", i=nk),
                ALU.mult)
            sm = w // P - 1
            for i, kb in enumerate(kbs):
                for s in range(w // P):
                    lhsT = (emq[:, i * P:(i + 1) * P] if s == sm
                            else ee[:, i * w + s * P:i * w + (s + 1) * P])
                    av(yq, s, kb, lhsT, h,
                       kb == 0 and s == 0, kb == 15 and s == 0)
        yb = h // 4
        rr4 = ptmp2.tile([P, 4], F32, name="rr4", tag="rr")
        yqv = yq[:].rearrange("p (qb d) -> p qb d", qb=4)
        nc.vector.reciprocal(rr4[:], yqv[:, :, NOPE])
        for qb in range(4):
            base = qb * NO1
            key = (yb, qb)
            if key not in yt4:
                yt4[key] = pq2.tile([P, P], BF16, name=f"yt4_{yb}_{qb}")
            nc.vector.tensor_scalar(
                yt4[key][:, (h % 4) * NOPE:(h % 4 + 1) * NOPE],
                yq[:, base:base + NOPE], rr4[:, qb:qb + 1], None, op0=ALU.mult)
        if h % 4 == 3:
            for qb in range(4):
                tp = pps.tile([P, P], BF16, name="ytp", tag="qsm", bufs=2)
                nc.tensor.transpose(tp[:], yt4[(yb, qb)][:], ident_bf[:])
                nc.vector.tensor_scalar(
                    yall[yb // 2][:, yb % 2, qb * P:(qb + 1) * P],
                    tp[:], 0.0, None, op0=ALU.add)

    # ---- Q projection + rope, interleaved with per-head attention
    # (1-mb lookahead: Q for mb+1 is emitted before heads of mb)
    qbf = []

    def q_proj(mb):
        tl = pq1.tile([P, TLOC], BF16, name=f"qbf{mb}")
        for ch in range(2):
            csl = slice(ch * CHUNK, (ch + 1) * CHUNK)
            ps = pps.tile([P, CHUNK], F32, name="qps", tag="qsm", bufs=2)
            for dcb in range(4):
                nc.tensor.matmul(ps[:], wq8_sb[:, dcb, :, mb * P:(mb + 1) * P],
                                 xl8_t[:, 2 * dcb:2 * dcb + 2, csl],
                                 start=(dcb == 0), stop=(dcb == 3),
                                 perf_mode=DR)
            nc.vector.tensor_tensor(tl[:, csl], ps[:], bc1l[:, csl], ALU.mult)
            qr = ptmp1.tile([2 * ROPE, CHUNK], BF16, name="qr", tag="qr")
            nc.vector.tensor_tensor(qr[0:ROPE, :], ps[32:64, :],
                                    bc1l[32:64, csl], ALU.mult)
            nc.vector.tensor_tensor(qr[ROPE:2 * ROPE, :], ps[96:128, :],
                                    bc1l[96:128, csl], ALU.mult)
            par = pps.tile([2 * ROPE, CHUNK], F32, name="qpar", tag="qsm", bufs=2)
            nc.tensor.matmul(par[:], perm64_sb[:], qr[:])
            t1 = ptmp1.tile([2 * ROPE, CHUNK], BF16, name="qt1", tag="qt1")
            nc.gpsimd.tensor_tensor(t1[:], qr[:], cosq_sb[:, csl], ALU.mult)
            t2 = ptmp1.tile([2 * ROPE, CHUNK], BF16, name="qt2", tag="qt2")
            nc.vector.tensor_tensor(t2[:], par[:], ssinq_sb[:, csl], ALU.mult)
            nc.gpsimd.tensor_tensor(tl[32:64, csl], t1[0:ROPE, :],
                                    t2[0:ROPE, :], ALU.add)
            nc.vector.tensor_tensor(tl[96:128, csl], t1[ROPE:2 * ROPE, :],
                                    t2[ROPE:2 * ROPE, :], ALU.add)
        qbf.append(tl)

    q_proj(0)
    emit_vext(range(0, 8))
    q_proj(1)
    emit_vext(range(8, NTB))
    for mb in range(8):
        if mb + 2 < 8:
            q_proj(mb + 2)
        attn_head(2 * mb)
        attn_head(2 * mb + 1)

    early.close()
    pat1 = whole.enter_context(tc.tile_pool(name="pat1", bufs=1, side="right"))
    pmx  = whole.enter_context(tc.tile_pool(name="pmx", bufs=1, side="right"))
    pwdw = whole.enter_context(tc.tile_pool(name="wdw", bufs=1, side="right"))
    wd_sb = []
    for idx in range(E + 1):
        tl = pwdw.tile([P, 2, 2, C], F8, name=f"wd8s{idx}")
        qeng[idx % 4].dma_start(tl[:], wd8.ap()[idx * P:(idx + 1) * P, :])
        wd_sb.append(tl)

    # ---- Wo + residual -> xa^T (kept in f32 to the end)
    xa = []
    for cb in range(NCB):
        xa.append(pat1.tile([P, TLOC], F32, name=f"xa{cb}"))
    sps2 = pacc.tile([1, TLOC], F32, name="ssq2", tag="accA")
    for cb in range(NCB):
        for ch in range(2):
            csl = slice(ch * CHUNK, (ch + 1) * CHUNK)
            ps = pps.tile([P, CHUNK], F32, name="ops", tag="qsm", bufs=2)
            for dib in range(2):
                nc.tensor.matmul(ps[:], wo8_sb[:, dib, :, cb * P:(cb + 1) * P],
                                 yall[dib][:, :, csl],
                                 start=(dib == 0), stop=(dib == 1),
                                 perf_mode=DR)
            nc.vector.scalar_tensor_tensor(xa[cb][:, csl], ps[:], 1.0 / WSCALE,
                                           xloc[cb][:, csl],
                                           op0=ALU.mult, op1=ALU.add)
        xq = ptmp2.tile([P, TLOC], BF16, name="xsq2", tag="xsqB")
        nc.scalar.activation(xq[:], xa[cb][:], AF.Square)
        nc.tensor.matmul(sps2[:], ones_128x1[:], xq[:],
                         start=(cb == 0), stop=(cb == NCB - 1))
    if DEBUG:
        for cb in range(NCB):
            nc.sync.dma_start(dbg["d_xaT"].ap()[cb * P:(cb + 1) * P, :],
                              xa[cb][:])

    # ---- rmsnorm2 + xmoe (MoE-phase pool pmx)
    invr2 = pmx.tile([1, TLOC], F32)
    rr2 = ptmp2.tile([1, TLOC], F32, name="rms2", tag="rmsB")
    nc.scalar.activation(rr2[:], sps2[:], AF.Sqrt, bias=eps_sb[:], scale=1.0 / C)
    nc.vector.reciprocal(invr2[:], rr2[:])
    if DEBUG:
        nc.sync.dma_start(dbg["d_invr2"].ap(), invr2[:])
    bc2 = pmx.tile([P, TLOC], F32)
    nc.gpsimd.partition_broadcast(bc2[:], invr2[:])
    # normalized MoE input, fp8, DoubleRow pair layout: [:, j, :] = chan
    # block 2*dcb+j
    xmoe8 = []
    for dcb in range(4):
        tl = pmx.tile([P, 2, TLOC], F8, name=f"xmoe8_{dcb}")
        nc.gpsimd.tensor_tensor(tl[:, 0, :], xa[2 * dcb][:], bc2[:], ALU.mult)
        nc.vector.tensor_tensor(tl[:, 1, :], xa[2 * dcb + 1][:], bc2[:],
                                ALU.mult)
        xmoe8.append(tl)

    # ---- gate (fp32)
    bcomb = [pmx.tile([P, TLOC], BF16, name=f"bcomb{e}") for e in range(E)]
    for tb in range(4):
        tsl = slice(tb * P, (tb + 1) * P)
        g_ps = pps.tile([P, E], F32, name="gps", tag="sps", bufs=2)
        for cb in range(NCB):
            nc.tensor.matmul(g_ps[:], xa[cb][:, tsl], wgate_sb[:, cb, :],
                             start=(cb == 0), stop=(cb == NCB - 1))
        ir_ps = pps.tile([P, 1], F32, name="irps", tag="sps", bufs=2)
        nc.tensor.transpose(ir_ps[:], invr2[:, tsl], ones1f[:])
        ir_col = ptmp2.tile([P, 1], F32, name="ircol", tag="ircol")
        nc.scalar.copy(ir_col[:], ir_ps[:])
        lg = ptmp2.tile([P, E], F32, name="lg", tag="lg")
        nc.vector.scalar_tensor_tensor(lg[:], g_ps[:], ir_col[:], biasg_sb[:],
                                       op0=ALU.mult, op1=ALU.add)
        m1 = ptmp2.tile([P, 1], F32, name="m1", tag="m1")
        nc.vector.reduce_max(m1[:], lg[:], axis=mybir.AxisListType.X)
        eq1 = ptmp2.tile([P, E], F32, name="eq1", tag="eq1")
        nc.vector.tensor_scalar(eq1[:], lg[:], m1[:], None, op0=ALU.is_equal)
        lm = ptmp2.tile([P, E], F32, name="lm", tag="lm")
        nc.vector.scalar_tensor_tensor(lm[:], eq1[:], -1e9, lg[:],
                                       op0=ALU.mult, op1=ALU.add)
        m2 = ptmp2.tile([P, 1], F32, name="m2", tag="m2")
        nc.vector.reduce_max(m2[:], lm[:], axis=mybir.AxisListType.X)
        eq2 = ptmp2.tile([P, E], F32, name="eq2", tag="eq2")
        nc.vector.tensor_scalar(eq2[:], lm[:], m2[:], None, op0=ALU.is_equal)
        dm = ptmp2.tile([P, 1], F32, name="dm", tag="dm")
        nc.vector.tensor_scalar(dm[:], m1[:], m2[:], None, op0=ALU.subtract)
        w1 = ptmp2.tile([P, 1], F32, name="w1", tag="w1")
        nc.scalar.activation(w1[:], dm[:], AF.Sigmoid)
        w2 = ptmp2.tile([P, 1], F32, name="w2", tag="w2")
        nc.vector.tensor_scalar(w2[:], w1[:], -1.0, 1.0, op0=ALU.mult,
                                op1=ALU.add)
        cmb = ptmp2.tile([P, E], F32, name="cmb", tag="cmb")
        nc.vector.tensor_scalar(cmb[:], eq1[:], w1[:], HSCALE, op0=ALU.mult,
                                op1=ALU.mult)
        cm2 = ptmp2.tile([P, E], F32, name="cm2", tag="cm2")
        nc.vector.tensor_scalar(cm2[:], eq2[:], w2[:], HSCALE, op0=ALU.mult,
                                op1=ALU.mult)
        cmf = ptmp2.tile([P, E], F32, name="cmf", tag="cmf")
        nc.vector.tensor_tensor(cmf[:], cmb[:], cm2[:], ALU.add)
        if DEBUG:
            nc.sync.dma_start(dbg["d_comb"].ap()[:, tb * E:(tb + 1) * E],
                              cmf[:])
        ct_ps = pps.tile([E, P], F32, name="ctps", tag="sps", bufs=2)
        nc.tensor.transpose(ct_ps[:], cmf[:], ident_sb[:])
        ct_sb = ptmp2.tile([E, P], BF16, name="ctsb", tag="ctsb")
        nc.scalar.copy(ct_sb[:], ct_ps[:])
        for e in range(E):
            bc_ps = pps.tile([P, P], F32, name="bcps", tag="qsm", bufs=2)
            nc.tensor.matmul(bc_ps[:], sel8_sb[:, e * P:(e + 1) * P],
                             ct_sb[:])
            if e % 2 == 0:
                nc.scalar.copy(bcomb[e][:, tsl], bc_ps[:])
            else:
                nc.vector.tensor_scalar(bcomb[e][:, tsl], bc_ps[:], 0.0,
                                        None, op0=ALU.add)

    attn.close()

    # ---- MoE: fp8 DoubleRow matmuls, PSUM accumulation across all experts.
    # Expert 0 = shared (gating = HSCALE), experts 1..8 gated by bcomb
    # (already scaled by HSCALE).  All weights are pre-scaled by WSCALE;
    # compensations: silu scale=1/WSCALE, hh mult 1/WSCALE, final add
    # 1/(WSCALE*HSCALE).
    moe = contextlib.ExitStack()
    pw   = moe.enter_context(tc.tile_pool(name="wmoe", bufs=1))
    pgu  = moe.enter_context(tc.tile_pool(name="psG", bufs=3, space="PSUM"))
    pwd  = moe.enter_context(tc.tile_pool(name="psD", bufs=1, space="PSUM"))
    pmoe = moe.enter_context(tc.tile_pool(name="hmoe", bufs=3))
    ph8  = moe.enter_context(tc.tile_pool(name="h8p", bufs=1))

    NEXP = E + 1
    NCB_W = 5               # wd psum banks held through phase A
    h8 = [[ph8.tile([P, 2, TLOC], F8, name=f"h8_{idx}_{dib}")
           for dib in range(2)] for idx in range(NEXP)]
    wdps = [pwd.tile([P, TLOC], F32, name=f"wdps{cb}")
            for cb in range(NCB_W)]

    def wd_partial(idx, cbs, tiles):
        for cb, wt in zip(cbs, tiles):
            csl = slice(cb * P, (cb + 1) * P)
            for dib in range(2):
                nc.tensor.matmul(wt[:], wd_sb[idx][:, dib, :, csl],
                                 h8[idx][dib][:],
                                 start=(idx == 0 and dib == 0),
                                 stop=(idx == NEXP - 1 and dib == 1),
                                 perf_mode=DR)

    def expert_gu(idx):
        wg_sb = pw.tile([P, 4, 2, I], F8, name="wg8s", tag=f"wg{idx % 2}")
        nc.sync.dma_start(wg_sb[:], wg8.ap()[idx * P:(idx + 1) * P, :])
        wu_sb = pw.tile([P, 4, 2, I], F8, name="wu8s", tag=f"wu{idx % 2}")
        nc.sync.dma_start(wu_sb[:], wu8.ap()[idx * P:(idx + 1) * P, :])
        for ib in range(NIB):
            isl = slice(ib * P, (ib + 1) * P)
            gp = pgu.tile([P, TLOC], F32, name="gp", tag="psG")
            for dcb in range(4):
                nc.tensor.matmul(gp[:], wg_sb[:, dcb, :, isl], xmoe8[dcb][:],
                                 start=(dcb == 0), stop=(dcb == 3),
                                 perf_mode=DR)
            sg = pmoe.tile([P, TLOC], BF16, name="sg", tag="sg")
            nc.scalar.activation(sg[:], gp[:], AF.Silu, scale=1.0 / WSCALE)
            up = pgu.tile([P, TLOC], F32, name="up", tag="psG")
            for dcb in range(4):
                nc.tensor.matmul(up[:], wu_sb[:, dcb, :, isl], xmoe8[dcb][:],
                                 start=(dcb == 0), stop=(dcb == 3),
                                 perf_mode=DR)
            hh = pmoe.tile([P, TLOC], BF16, name="hh", tag="hh")
            nc.vector.scalar_tensor_tensor(hh[:], up[:], 1.0 / WSCALE, sg[:],
                                           op0=ALU.mult, op1=ALU.mult)
            dst = h8[idx][ib // 2][:, ib % 2, :]
            if idx == 0:
                nc.gpsimd.tensor_scalar(dst, hh[:], HSCALE, None, op0=ALU.mult)
            else:
                nc.gpsimd.tensor_tensor(dst, hh[:], bcomb[idx - 1][:],
                                        ALU.mult)

    # phase A with 1-expert-lagged partial Wd for cb < NCB_W
    for idx in range(NEXP):
        expert_gu(idx)
        if idx > 1:
            wd_partial(idx - 2, range(NCB_W), wdps)
    wd_partial(NEXP - 2, range(NCB_W), wdps)
    wd_partial(NEXP - 1, range(NCB_W), wdps)
    for cb in range(NCB_W):
        fo = pmoe.tile([P, TLOC], F32, name="fo", tag="fo")
        nc.vector.scalar_tensor_tensor(fo[:], wdps[cb][:],
                                       1.0 / (WSCALE * HSCALE),
                                       xa[cb][:], op0=ALU.mult, op1=ALU.add)
        nc.sync.dma_start(outT.ap()[cb * P:(cb + 1) * P, :], fo[:])
    # tail: remaining cb reuse the freed g/u psum slots
    for cb in range(NCB_W, NCB):
        wt = pgu.tile([P, TLOC], F32, name="wdtail", tag="psG")
        for idx in range(NEXP):
            csl = slice(cb * P, (cb + 1) * P)
            for dib in range(2):
                nc.tensor.matmul(wt[:], wd_sb[idx][:, dib, :, csl],
                                 h8[idx][dib][:],
                                 start=(idx == 0 and dib == 0),
                                 stop=(idx == NEXP - 1 and dib == 1),
                                 perf_mode=DR)
        fo = pmoe.tile([P, TLOC], F32, name="fo", tag="fo")
        nc.vector.scalar_tensor_tensor(fo[:], wt[:],
                                       1.0 / (WSCALE * HSCALE),
                                       xa[cb][:], op0=ALU.mult, op1=ALU.add)
        nc.sync.dma_start(outT.ap()[cb * P:(cb + 1) * P, :], fo[:])

    moe.close()
    whole.close()


# =============================================================== host side
def _build():
    if "nc" in _CACHE:
        return _CACHE["nc"]
    nc = bacc.Bacc("TRN2", target_bir_lowering=False, debug=False,
                   num_devices=8)
    with tile.TileContext(nc) as tc:
        _emit(nc, tc)
    nc.compile()
    _CACHE["nc"] = nc
    return nc


def _rope_tables(pos):
    # pos: (N,) positions; returns cos,ssin of shape (ROPE, N) in the
    # row-pair layout (rows 2i/2i+1 both carry angle pos*freq_i; ssin row 2i
    # is -sin, row 2i+1 is +sin).
    freqs = 1.0 / (THETA ** (np.arange(0, ROPE, 2, dtype=np.float32) / ROPE))
    ang = np.outer(freqs, pos.astype(np.float32))          # (16, N)
    cos = np.repeat(np.cos(ang), 2, axis=0).astype(np.float32)
    sin = np.sin(ang).astype(np.float32)
    ssin = np.empty((ROPE, len(pos)), np.float32)
    ssin[0::2] = -sin
    ssin[1::2] = sin
    return cos, ssin


def _host_inputs(inputs, core):
    bf = lambda a: np.ascontiguousarray(a).astype(ml_dtypes.bfloat16)
    f32 = lambda a: np.ascontiguousarray(a, dtype=np.float32)
    b, q = core // 4, core % 4
    qb_s = [15 - 4 * s - q for s in range(4)]   # query block per slot
    x = np.asarray(inputs["x"], np.float32)
    w_ln1 = np.asarray(inputs["w_ln1"], np.float32)
    w_ln2 = np.asarray(inputs["w_ln2"], np.float32)
    xT = x[b].T                                            # (C, T)
    loc_cols = np.concatenate(
        [np.arange(j * P, (j + 1) * P) for j in qb_s])
    xloc = xT[:, loc_cols]

    # rope tables
    posq = loc_cols.astype(np.float32)
    cq, sq = _rope_tables(posq)
    cosq = np.vstack([cq, cq])                             # (64, 512)
    ssinq = np.vstack([sq, sq])
    posk = np.arange(T, dtype=np.float32)
    coskp, ssinkp = _rope_tables(posk)           # (32, 2048) = [32, nt*512+t]

    # permutation matrices (pair swap)
    p32 = np.zeros((ROPE, ROPE), np.float32)
    for i in range(ROPE // 2):
        p32[2 * i + 1, 2 * i] = 1.0
        p32[2 * i, 2 * i + 1] = 1.0
    p64 = np.zeros((2 * ROPE, 2 * ROPE), np.float32)
    p64[:ROPE, :ROPE] = p32
    p64[ROPE:, ROPE:] = p32

    # causal mask for the (single) masked slot at each key block: at kb the
    # masked slot is s = 3 - kb//4 with query block qb_s[s]
    kmask = np.zeros((P, NTB * P), np.float32)
    ki = np.arange(P)[:, None]
    qi = np.arange(P)[None, :]
    for kb in range(NTB):
        j = qb_s[3 - kb // 4]
        if kb < j:
            m = np.ones((P, P), np.float32)
        elif kb == j:
            m = (ki <= qi).astype(np.float32)
        else:
            m = np.zeros((P, P), np.float32)
        kmask[:, kb * P:(kb + 1) * P] = m

    wq = np.asarray(inputs["Wq"], np.float32) * w_ln1[:, None]
    wkva = np.asarray(inputs["Wkva"], np.float32) * w_ln1[:, None]
    wo_nope = np.asarray(inputs["Wo"], np.float32).reshape(H, D, C)[:, :NOPE]
    wgate = np.asarray(inputs["Wgate"], np.float32) * w_ln2[:, None]
    biasg = np.broadcast_to(np.asarray(inputs["expert_bias"], np.float32),
                            (P, E)).copy()

    # fp8 expert weights with expert 0 = shared; DoubleRow pair packing.
    fp8 = ml_dtypes.float8_e4m3
    wg_all = np.concatenate(
        [np.asarray(inputs["sWg"], np.float32)[0:1],
         np.asarray(inputs["Wg"], np.float32)], axis=0) * w_ln2[None, :, None]
    wu_all = np.concatenate(
        [np.asarray(inputs["sWu"], np.float32)[0:1],
         np.asarray(inputs["Wu"], np.float32)], axis=0) * w_ln2[None, :, None]
    wd_all = np.concatenate(
        [np.asarray(inputs["sWd"], np.float32)[0:1],
         np.asarray(inputs["Wd"], np.float32)], axis=0)

    def pack_pairs(w, nblk):
        # w: (NEXP, K, M) with K = 256*nblk -> (NEXP*128, nblk*2*M):
        # row e*128+k, col [blk, j, m] = w[e, 256*blk + 128*j + k, m]
        ne, kk, mm = w.shape
        assert kk == 256 * nblk
        r = w.reshape(ne, nblk, 2, P, mm).transpose(0, 3, 1, 2, 4)
        return np.ascontiguousarray(r.reshape(ne * P, nblk * 2 * mm))

    fp8c = lambda a: np.clip(a, -240, 240).astype(ml_dtypes.float8_e4m3)
    m = {
        "xT8": fp8c(xT),
        "xlocT": f32(xloc),
        "xloc8": fp8c(xloc),
        "wq8": pack_pairs(wq[None] * WSCALE, 4).astype(ml_dtypes.float8_e4m3),
        "wkva8": pack_pairs(wkva[None] * WSCALE, 4).astype(ml_dtypes.float8_e4m3),
        "wkvb": bf(inputs["Wkvb"]),
        "wo8": pack_pairs(wo_nope.reshape(H * NOPE, C)[None]
                          * WSCALE, 2).astype(fp8),
        "cosq": bf(cosq), "ssinq": bf(ssinq),
        "coskp": bf(coskp), "ssinkp": bf(ssinkp),
        "perm64": bf(p64), "perm32": bf(p32),
        "ident": np.eye(P, dtype=np.float32),
        "identbf": np.eye(P, dtype=ml_dtypes.bfloat16),
        "sel8": np.kron(np.eye(E, dtype=np.float32),
                        np.ones((1, P), np.float32)).astype(ml_dtypes.bfloat16),
        "kmask": bf(kmask),
        "wgate": f32(wgate),
        "biasg": biasg,
        "wg8": pack_pairs(wg_all * WSCALE, 4).astype(fp8),
        "wu8": pack_pairs(wu_all * WSCALE, 4).astype(fp8),
        "wd8": pack_pairs(wd_all * WSCALE, 2).astype(fp8),
    }
    return m


def kernel(**inputs):
    nc = _build()
    in_maps = [_host_inputs(inputs, core) for core in range(8)]
    kw = {}
    if os.environ.get("BASSK_TRACE"):
        kw = dict(trace=True, tmpdir=os.environ.get("BASSK_TRACE_DIR") or None)
    res = bass_utils.run_bass_kernel_spmd(nc, in_maps, core_ids=list(range(8)),
                                          **kw)
    if os.environ.get("BASSK_TRACE"):
        print(f"PROFILE exec_time_ns: {res.exec_time_ns}")
        print(f"PROFILE json: {res.profile_json}")
    out = np.empty((B, T, C), np.float32)
    for core in range(8):
        b, q = core // 4, core % 4
        oT = res.results[core]["outT"]                     # (C, 512)
        for s in range(4):
            j = 15 - 4 * s - q
            out[b, j * P:(j + 1) * P] = oT[:, s * P:(s + 1) * P].T
    return out



# revision 81
# speedup vs baseline: 1.0743x; 1.0280x over previous
# DeepSeek block (MLA attention + top-2-of-8 MoE + shared expert) on 8 TRN2
# NeuronCores, zero-collective sharding.
#
# Core c in [0..8): sequence b = c//4, q = c%4; owns token chunks
# hi = 7-q (slot 0) and lo = q (slot 1), 256 tokens each (causally balanced:
# every core's true causal work is 9 key blocks of 256).
#
# The SPMD program is identical on all cores; everything core-specific
# arrives as data: slot 0 attends key blocks [0..16), slot 1 [0..8), with
# host-built multiplicative masks (ones for fully-past blocks, triangular at
# the causal boundary, zeros for fully-future blocks).
#
# Layout: activations live feature-on-partition ("^T", tokens on the free
# axis). Host pre-transposes x, folds w_ln1/w_ln2 into adjacent weights,
# pre-casts weights to bf16, and builds rope tables / masks / identity.
# Matmuls run in bf16 (fp32 PSUM accumulate); the gate runs in fp32.
import os
import numpy as np
import ml_dtypes

import concourse.bacc as bacc
import concourse.mybir as mybir
import concourse.tile as tile
from concourse import bass_utils

F32 = mybir.dt.float32
F32R = mybir.dt.float32r
BF16 = mybir.dt.bfloat16
F8 = mybir.dt.float8e4
DR = mybir.MatmulPerfMode.DoubleRow
AF = mybir.ActivationFunctionType
ALU = mybir.AluOpType

WSCALE = 64.0            # fp8 expert-weight scale (keeps values normal-range)
HSCALE = 16.0            # fp8 h-activation scale (folded into comb weights)

B, T, C, H, D = 2, 2048, 1024, 16, 64
R, ROPE, NOPE = 128, 32, 32
E, I = 8, 512
THETA, EPS = 100000.0, 1e-5
P = 128
NCB = C // P             # 8 C blocks
NTB = T // P             # 16 key/token blocks per sequence
TLOC, CHUNK = 512, 256
KB_SLOT = (16, 8)        # key blocks attended per chunk slot
NKB = sum(KB_SLOT)
NIB = I // P             # 4 I blocks

DEBUG = bool(int(os.environ.get("BASSK_DEBUG", "0")))
_CACHE = {}


# =============================================================== device IR
def _emit(nc, tc):
    import contextlib

    def din(name, shape, dt):
        return nc.dram_tensor(name, shape, dt, kind="ExternalInput")

    xT8    = din("xT8", (C, T), F8)
    xlocT  = din("xlocT", (C, TLOC), F32)
    xloc8  = din("xloc8", (C, TLOC), F8)
    wq8    = din("wq8", (P, 4 * 2 * H * D), F8)    # DoubleRow pairs, x WSCALE
    wkva8  = din("wkva8", (P, 4 * 2 * (R + ROPE)), F8)
    wkvb   = din("wkvb", (R, H * NOPE), BF16)
    wo8    = din("wo8", (P, 2 * 2 * C), F8)   # DR pairs, x WSCALE
    cosq   = din("cosq", (2 * ROPE, TLOC), BF16)
    ssinq  = din("ssinq", (2 * ROPE, TLOC), BF16)
    coskp  = din("coskp", (ROPE, 4 * 512), BF16)  # [32, nt*512+t]
    ssinkp = din("ssinkp", (ROPE, 4 * 512), BF16)
    perm64 = din("perm64", (2 * ROPE, 2 * ROPE), BF16)
    perm32 = din("perm32", (ROPE, ROPE), BF16)
    ident  = din("ident", (P, P), F32)
    identbf = din("identbf", (P, P), BF16)
    sel8   = din("sel8", (E, E * P), BF16)   # sel8[k, e*128+m] = (k == e)
    kmask  = din("kmask", (P, NTB * P), BF16)
    wgate  = din("wgate", (C, E), F32)
    biasg  = din("biasg", (P, E), F32)
    # fp8 expert weights, expert 0 = shared.  wg8/wu8 rows e*P+k hold
    # [dcb, j, m] -> W[256*dcb + 128*j + k, m] * WSCALE (DoubleRow pairs on j).
    # wd8 rows e*P+k hold [dib, j, c] -> Wd[256*dib + 128*j + k, c] * WSCALE.
    wg8    = din("wg8", ((E + 1) * P, 4 * 2 * I), F8)
    wu8    = din("wu8", ((E + 1) * P, 4 * 2 * I), F8)
    wd8    = din("wd8", ((E + 1) * P, 2 * 2 * C), F8)

    outT = nc.dram_tensor("outT", (C, TLOC), F32, kind="ExternalOutput")
    dbg = {}
    if DEBUG:
        for name, shape in [("d_xaT", (C, TLOC)), ("d_comb", (P, 4 * E)),
                            ("d_invr1", (1, T)), ("d_invr2", (1, TLOC))]:
            dbg[name] = nc.dram_tensor(name, shape, F32, kind="ExternalOutput")

    f32r = lambda ap: ap.bitcast(F32R)

    # ---------------- pools (sized to fit 192KB/partition SBUF, 8 PSUM banks)
    whole = contextlib.ExitStack()   # whole kernel
    attn  = contextlib.ExitStack()   # until gate done
    early = contextlib.ExitStack()   # until Q/ckv/krope done
    xtst  = contextlib.ExitStack()   # xt tiles, until ckv done
    pc   = whole.enter_context(tc.tile_pool(name="pc", bufs=1))
    pq1  = attn.enter_context(tc.tile_pool(name="pq1", bufs=1))
    pkv0 = attn.enter_context(tc.tile_pool(name="pkv0", bufs=1))
    pps  = attn.enter_context(tc.tile_pool(name="psA", bufs=2, space="PSUM"))
    pacc = attn.enter_context(tc.tile_pool(name="psB", bufs=2, space="PSUM"))
    px   = early.enter_context(tc.tile_pool(name="px", bufs=1, side="right"))
    ptmp1 = early.enter_context(tc.tile_pool(name="tmp1", bufs=2, side="right"))
    pxt  = xtst.enter_context(tc.tile_pool(name="pxt", bufs=1, side="right"))

    # ---- constants
    ones_128x1 = pc.tile([P, 1], BF16); nc.any.memset(ones_128x1[:], 1.0)
    ones1f = pc.tile([1, 1], F32); nc.any.memset(ones1f[:], 1.0)
    eps_sb = pc.tile([1, 1], F32); nc.any.memset(eps_sb[:], EPS)
    eps64 = pc.tile([1, 1], F32)
    nc.any.memset(eps64[:], EPS * WSCALE * WSCALE)
    ident_sb = pc.tile([P, P], F32); nc.sync.dma_start(ident_sb[:], ident.ap())
    ident_bf = pc.tile([P, P], BF16)
    nc.sync.dma_start(ident_bf[:], identbf.ap())
    sel8_sb = pc.tile([E, E * P], BF16)
    nc.sync.dma_start(sel8_sb[:], sel8.ap())
    perm64_sb = pc.tile([2 * ROPE, 2 * ROPE], BF16)
    nc.sync.dma_start(perm64_sb[:], perm64.ap())
    perm32_sb = pc.tile([ROPE, ROPE], BF16)
    nc.sync.dma_start(perm32_sb[:], perm32.ap())
    biasg_sb = pc.tile([P, E], F32); nc.sync.dma_start(biasg_sb[:], biasg.ap())
    cosq_sb = px.tile([2 * ROPE, TLOC], BF16)
    nc.sync.dma_start(cosq_sb[:], cosq.ap())
    ssinq_sb = px.tile([2 * ROPE, TLOC], BF16)
    nc.sync.dma_start(ssinq_sb[:], ssinq.ap())
    coskp_all = px.tile([ROPE, 4, 512], BF16, name="coskp_all")
    nc.sync.dma_start(coskp_all[:], coskp.ap())
    coskp_sb = [coskp_all[:, nt, :] for nt in range(4)]
    ssinkp_all = px.tile([ROPE, 4, 512], BF16, name="ssinkp_all")
    nc.gpsimd.dma_start(ssinkp_all[:], ssinkp.ap())
    ssinkp_sb = [ssinkp_all[:, nt, :] for nt in range(4)]

    # ---- bulk loads (early), DMA issues spread across engine queues
    qeng = [nc.sync, nc.scalar, nc.sync, nc.gpsimd, nc.sync, nc.scalar]
    def load_blocks(pool, name, dram_ap, nblk, width, dt, nsplit, qoff=0):
        # one [P, nblk, width] tile, DMA'd in nsplit chunks across queues
        tl = pool.tile([P, nblk, width], dt, name=name)
        per = nblk // nsplit
        src_v = dram_ap.rearrange("(cb p) f -> p cb f", p=P)
        for s in range(nsplit):
            qeng[(qoff + s) % 6].dma_start(
                tl[:, s * per:(s + 1) * per, :],
                src_v[:, s * per:(s + 1) * per, :])
        return [tl[:, cb, :] for cb in range(nblk)]

    xt_t = pxt.tile([P, NCB, T], F8, name="xt8_all")
    for s in range(NCB):
        qeng[s % 6].dma_start(
            xt_t[:, s:s + 1, :],
            xT8.ap().rearrange("(cb p) f -> p cb f", p=P)[:, s:s + 1, :])
    xt = [xt_t[:, cb, :] for cb in range(NCB)]
    xloc = load_blocks(pq1, "xloc_all", xlocT.ap(), NCB, TLOC, F32, 2, 4)
    xl8_t = px.tile([P, NCB, TLOC], F8, name="xl8_all")
    nc.scalar.dma_start(xl8_t[:],
                        xloc8.ap().rearrange("(cb p) f -> p cb f", p=P))
    wq8_sb = px.tile([P, 4, 2, H * D], F8, name="wq8_sb")
    for s in range(2):
        qeng[s].dma_start(wq8_sb[:, 2 * s:2 * s + 2, :, :],
                          wq8.ap().rearrange("p (d j f) -> p d j f", d=4, j=2)
                          [:, 2 * s:2 * s + 2, :, :])
    wkva8_sb = px.tile([P, 4, 2, R + ROPE], F8, name="wkva8_sb")
    nc.gpsimd.dma_start(wkva8_sb[:],
                        wkva8.ap().rearrange("p (d j f) -> p d j f", d=4, j=2))
    wkvb_sb = pkv0.tile([R, H * NOPE], BF16, name="wkvbs")
    nc.gpsimd.dma_start(wkvb_sb[:], wkvb.ap())

    # ---- rmsnorm1 stats: global (keys) then local (queries)
    invr1 = px.tile([1, T], BF16)
    bc1 = px.tile([P, T], BF16)
    for nt in range(T // 512):
        sl = slice(nt * 512, (nt + 1) * 512)
        sps = pacc.tile([1, 512], F32, name="ssq1", tag="accA")
        for cb in range(NCB):
            xq = ptmp1.tile([P, 512], BF16, name="xsq", tag="xsq")
            if cb % 2 == 0:
                nc.scalar.activation(xq[:], xt[cb][:, sl], AF.Square)
            else:
                nc.vector.tensor_tensor(xq[:], xt[cb][:, sl], xt[cb][:, sl],
                                        ALU.mult)
            nc.tensor.matmul(sps[:], ones_128x1[:], xq[:],
                             start=(cb == 0), stop=(cb == NCB - 1))
        rr = ptmp1.tile([1, 512], F32, name="rms1", tag="rms1")
        nc.scalar.activation(rr[:], sps[:], AF.Sqrt, bias=eps64[:],
                             scale=WSCALE * WSCALE / C)
        with nc.allow_low_precision(reason="rmsnorm scale in bf16"):
            nc.vector.reciprocal(invr1[:, sl], rr[:])
        nc.gpsimd.partition_broadcast(bc1[:, sl], invr1[:, sl])


    # ---- ckv (scaled);  k_rope raw kept packed in two (64,512) tiles
    kvlat = pkv0.tile([R, T], BF16)
    kropef = [px.tile([ROPE, 512], BF16, name=f"kropef{i}") for i in range(4)]
    for nt in range(T // 512):
        sl = slice(nt * 512, (nt + 1) * 512)
        lat_ps = pps.tile([P, 512], F32, name="latps", tag="sps", bufs=2)
        rop_ps = pps.tile([ROPE, 512], F32, name="ropps", tag="sps", bufs=2)
        for dcb in range(4):
            nc.tensor.matmul(lat_ps[:], wkva8_sb[:, dcb, :, 0:R],
                             xt_t[:, 2 * dcb:2 * dcb + 2, sl],
                             start=(dcb == 0), stop=(dcb == 3), perf_mode=DR)
        for dcb in range(4):
            nc.tensor.matmul(rop_ps[:], wkva8_sb[:, dcb, :, R:R + ROPE],
                             xt_t[:, 2 * dcb:2 * dcb + 2, sl],
                             start=(dcb == 0), stop=(dcb == 3), perf_mode=DR)
        nc.vector.tensor_tensor(kvlat[:, sl], lat_ps[:], bc1[:, sl], ALU.mult)
        nc.vector.tensor_tensor(kropef[nt][:], rop_ps[:],
                                bc1[0:ROPE, sl], ALU.mult)

    invr1l = px.tile([1, TLOC], F32)
    bc1l = px.tile([P, TLOC], F32)
    spsl = pacc.tile([1, TLOC], F32, name="ssql", tag="accA")
    for cb in range(NCB):
        xq = ptmp1.tile([P, TLOC], BF16, name="xsql", tag="xsq")
        nc.scalar.activation(xq[:], xloc[cb][:], AF.Square)
        nc.tensor.matmul(spsl[:], ones_128x1[:], xq[:],
                         start=(cb == 0), stop=(cb == NCB - 1))
    rrl = ptmp1.tile([1, TLOC], F32, name="rmsl", tag="rms1")
    nc.scalar.activation(rrl[:], spsl[:], AF.Sqrt, bias=eps64[:],
                         scale=WSCALE * WSCALE / C)
    nc.vector.reciprocal(invr1l[:], rrl[:])
    nc.gpsimd.partition_broadcast(bc1l[:], invr1l[:])

    # ---- rope K -> kr32 (32, T)
    kr32 = pkv0.tile([ROPE, T], BF16)
    for nt in range(T // 512):
        sl = slice(nt * 512, (nt + 1) * 512)
        src = kropef[nt][:]
        par_ps = pps.tile([ROPE, 512], F32, name="parps", tag="sps", bufs=2)
        nc.tensor.matmul(par_ps[:], perm32_sb[:], src)
        t1 = ptmp1.tile([ROPE, 512], BF16, name="kr1", tag="kr1")
        nc.vector.tensor_tensor(t1[:], src, coskp_sb[nt][:], ALU.mult)
        t2 = ptmp1.tile([ROPE, 512], BF16, name="kr2", tag="kr2")
        nc.vector.tensor_tensor(t2[:], par_ps[:], ssinkp_sb[nt][:], ALU.mult)
        nc.vector.tensor_tensor(kr32[:, sl], t1[:], t2[:], ALU.add)

    xtst.close()
    pkv  = attn.enter_context(tc.tile_pool(name="pkv", bufs=1))
    pq2  = attn.enter_context(tc.tile_pool(name="pq2", bufs=1))
    ptmp2 = attn.enter_context(tc.tile_pool(name="tmp2", bufs=2))
    pE   = attn.enter_context(tc.tile_pool(name="pE", bufs=4))

    # ---- k_nope (transient), V_ext, kfull (emitted before Q so the PE/Act/
    # DMA work here overlaps the DVE-heavy Q phase)
    sub = contextlib.ExitStack()
    pkx = sub.enter_context(tc.tile_pool(name="pkx", bufs=2))
    kfull = [pkv.tile([P, T], BF16, name=f"kfull{mb}") for mb in range(8)]
    for half in range(2):
        knope = []
        for mb in (2 * half, 2 * half + 1):
            tl = pkx.tile([P, T], BF16, name="knope", tag="kn")
            for nt in range(T // 512):
                sl = slice(nt * 512, (nt + 1) * 512)
                ps = pps.tile([P, 512], F32, name="knps", tag="sps", bufs=2)
                nc.tensor.matmul(ps[:], wkvb_sb[:, mb * P:(mb + 1) * P],
                                 kvlat[:, sl])
                if nt % 2 == 0:
                    nc.scalar.copy(tl[:, sl], ps[:])
                else:
                    nc.vector.tensor_scalar(tl[:, sl], ps[:], 0.0, None,
                                            op0=ALU.add)
            knope.append(tl)
        for mb in range(4 * half, 4 * half + 4):
            tl = kfull[mb]
            h0, h1 = 2 * mb, 2 * mb + 1
            e0, e1 = (nc.sync, nc.gpsimd) if mb % 2 else (nc.gpsimd, nc.sync)
            # nope rows: one DMA lane + one DVE tensor-copy lane (a (32,2048)
            # DVE copy is cheaper than its DMA and keeps the DMA burst short)
            e0.dma_start(tl[0:32, :],
                         knope[h0 // 4 - 2 * half]
                         [(h0 % 4) * 32:(h0 % 4) * 32 + 32, :])
            nc.vector.tensor_scalar(
                tl[64:96, :],
                knope[h1 // 4 - 2 * half][(h1 % 4) * 32:(h1 % 4) * 32 + 32, :],
                0.0, None, op0=ALU.add)
            e0.dma_start(tl[32:64, :], kr32[:])
            e1.dma_start(tl[96:128, :], kr32[:])
    sub.close()
    vext = [pkv.tile([P, H, 34], BF16, name=f"vext{tb}")
            for tb in range(NTB)]

    def emit_vext(tbs):
        for tb in tbs:
            tl = vext[tb]
            ps = pps.tile([P, H * NOPE], F32, name="vps", tag="sps", bufs=2)
            nc.tensor.matmul(ps[:], kvlat[:, tb * P:(tb + 1) * P], wkvb_sb[:])
            if tb % 2 == 0:
                nc.scalar.copy(tl[:, :, 0:NOPE],
                               ps[:].rearrange("p (h d) -> p h d", h=H))
            else:
                nc.vector.tensor_scalar(
                    tl[:, :, 0:NOPE],
                    ps[:].rearrange("p (h d) -> p h d", h=H),
                    0.0, None, op0=ALU.add)
            nc.any.memset(tl[:, :, NOPE:NOPE + 1], 1.0)

    # deferred loads for the scores/Wo/MoE phases (DMAs overlap attention)
    kmask_sb = pkv.tile([P, NTB * P], BF16)
    nc.gpsimd.dma_start(kmask_sb[:], kmask.ap())
    wo8_sb = pq2.tile([P, 2, 2, C], F8, name="wo8_sb")
    nc.sync.dma_start(wo8_sb[:],
                      wo8.ap().rearrange("p (d j f) -> p d j f", d=2, j=2))
    wgate_sb = pq2.tile([P, NCB, E], F32, name="wgate_sb")
    nc.gpsimd.dma_start(wgate_sb[:], wgate.ap().rearrange(
        "(cb p) e -> p cb e", p=P))

    # ---- attention core defs (q-major AV; exp in merged 512-wide tiles)
    # slot0 (q cols 0:256) attends kb 0..16, slot1 (cols 256:512) kb 0..8.
    # kb<8: one (128,512) score tile covers both slots; slot0 half needs no
    # mask (always fully-past), slot1 half gets kmask.  kb>=8 (slot0 only):
    # pairs (8+2p, 9+2p) share a (128,512) tile, masked as one 512-wide mult.
    # AV accumulates q-major: yq[:, qb*33:qb*33+33] = sum_k em[k,q] vext[k,:].
    yall = [pq2.tile([P, 2, TLOC], F8, name=f"yall{d}") for d in range(2)]
    yt4 = {}
    NO1 = NOPE + 1

    def av(yq, qb, kb, lhsT, h, start, stop):
        nc.tensor.matmul(yq[:, qb * NO1:(qb + 1) * NO1], lhsT,
                         vext[kb][:, h, 0:NO1], start=start, stop=stop)

    # striped decomposition: 4 query slots of 128 tokens with causal needs
    # (16, 12, 8, 4) key-blocks (identical on every core; masks are data).
    # q cols = [slot0 | slot1 | slot2 | slot3].  At iteration kb the active
    # width is (4 - kb//4)*128 and exactly the LAST active slot is masked.
    STILES = [([0, 1], 512), ([2, 3], 512), ([4, 5], 384), ([6, 7], 384),
              ([8, 9, 10, 11], 256), ([12, 13, 14, 15], 128)]

    def attn_head(h):
        mb, po = h // 2, (h % 2) * 64
        yq = pacc.tile([P, 4 * NO1], F32, name="yq", tag="accA")
        for ti, (kbs, w) in enumerate(STILES):
            nk = len(kbs)
            nn = nk * w
            subw = TLOC if w == 384 else w   # pad 384 to the bank boundary
            s_ps = pps.tile([P, nk, subw], F32, name="sps", tag="sps", bufs=2)
            for i, kb in enumerate(kbs):
                nc.tensor.matmul(s_ps[:, i, 0:w],
                                 kfull[mb][po:po + 64, kb * P:(kb + 1) * P],
                                 qbf[mb][po:po + 64, 0:w])
            ee = pE.tile([P, nn], BF16, name="ee", tag="ee", bufs=6)
            nc.scalar.activation(ee[:].rearrange("p (i c) -> p i c", i=nk),
                                 s_ps[:, :, 0:w], AF.Exp, scale=0.125)
            emq = pE.tile([P, nk * P], BF16, name="emq", tag="emq", bufs=2)
            eng = nc.gpsimd if ti % 2 == 0 else nc.vector
            eng.tensor_tensor(
                emq[:].rearrange("p (i c) -> p i c", i=nk),
                ee[:].rearrange("p (i c) -> p i c", i=nk)[:, :, w - P:w],
                kmask_sb[:, kbs[0] * P:(kbs[-1] + 1) * P]
                .rearrange("p (i c) -> p i c", i=nk),
                ALU.mult)
            sm = w // P - 1
            for i, kb in enumerate(kbs):
                for s in range(w // P):
                    lhsT = (emq[:, i * P:(i + 1) * P] if s == sm
                            else ee[:, i * w + s * P:i * w + (s + 1) * P])
                    av(yq, s, kb, lhsT, h,
                       kb == 0 and s == 0, kb == 15 and s == 0)
        yb = h // 4
        rr4 = ptmp2.tile([P, 4], F32, name="rr4", tag="rr")
        yqv = yq[:].rearrange("p (qb d) -> p qb d", qb=4)
        nc.vector.reciprocal(rr4[:], yqv[:, :, NOPE])
        for qb in range(4):
            base = qb * NO1
            key = (yb, qb)
            if key not in yt4:
                yt4[key] = pq2.tile([P, P], BF16, name=f"yt4_{yb}_{qb}")
            nc.vector.tensor_scalar(
                yt4[key][:, (h % 4) * NOPE:(h % 4 + 1) * NOPE],
                yq[:, base:base + NOPE], rr4[:, qb:qb + 1], None, op0=ALU.mult)
        if h % 4 == 3:
            for qb in range(4):
                tp = pps.tile([P, P], BF16, name="ytp", tag="qsm", bufs=2)
                nc.tensor.transpose(tp[:], yt4[(yb, qb)][:], ident_bf[:])
                nc.vector.tensor_scalar(
                    yall[yb // 2][:, yb % 2, qb * P:(qb + 1) * P],
                    tp[:], 0.0, None, op0=ALU.add)

    # ---- Q projection + rope, interleaved with per-head attention
    # (1-mb lookahead: Q for mb+1 is emitted before heads of mb)
    qbf = []

    def q_proj(mb):
        tl = pq1.tile([P, TLOC], BF16, name=f"qbf{mb}")
        for ch in range(2):
            csl = slice(ch * CHUNK, (ch + 1) * CHUNK)
            ps = pps.tile([P, CHUNK], F32, name="qps", tag="qsm", bufs=2)
            for dcb in range(4):
                nc.tensor.matmul(ps[:], wq8_sb[:, dcb, :, mb * P:(mb + 1) * P],
                                 xl8_t[:, 2 * dcb:2 * dcb + 2, csl],
                                 start=(dcb == 0), stop=(dcb == 3),
                                 perf_mode=DR)
            nc.vector.tensor_tensor(tl[:, csl], ps[:], bc1l[:, csl], ALU.mult)
            qr = ptmp1.tile([2 * ROPE, CHUNK], BF16, name="qr", tag="qr")
            nc.vector.tensor_tensor(qr[0:ROPE, :], ps[32:64, :],
                                    bc1l[32:64, csl], ALU.mult)
            nc.vector.tensor_tensor(qr[ROPE:2 * ROPE, :], ps[96:128, :],
                                    bc1l[96:128, csl], ALU.mult)
            par = pps.tile([2 * ROPE, CHUNK], F32, name="qpar", tag="qsm", bufs=2)
            nc.tensor.matmul(par[:], perm64_sb[:], qr[:])
            t1 = ptmp1.tile([2 * ROPE, CHUNK], BF16, name="qt1", tag="qt1")
            nc.gpsimd.tensor_tensor(t1[:], qr[:], cosq_sb[:, csl], ALU.mult)
            t2 = ptmp1.tile([2 * ROPE, CHUNK], BF16, name="qt2", tag="qt2")
            nc.vector.tensor_tensor(t2[:], par[:], ssinq_sb[:, csl], ALU.mult)
            nc.gpsimd.tensor_tensor(tl[32:64, csl], t1[0:ROPE, :],
                                    t2[0:ROPE, :], ALU.add)
            nc.vector.tensor_tensor(tl[96:128, csl], t1[ROPE:2 * ROPE, :],
                                    t2[ROPE:2 * ROPE, :], ALU.add)
        qbf.append(tl)

    q_proj(0)
    emit_vext(range(0, 8))
    q_proj(1)
    emit_vext(range(8, NTB))
    for mb in range(8):
        if mb + 2 < 8:
            q_proj(mb + 2)
        attn_head(2 * mb)
        attn_head(2 * mb + 1)

    early.close()
    pat1 = whole.enter_context(tc.tile_pool(name="pat1", bufs=1, side="right"))
    pmx  = whole.enter_context(tc.tile_pool(name="pmx", bufs=1, side="right"))
    pwdw = whole.enter_context(tc.tile_pool(name="wdw", bufs=1, side="right"))
    wd_sb = []
    for idx in range(E + 1):
        tl = pwdw.tile([P, 2, 2, C], F8, name=f"wd8s{idx}")
        qeng[idx % 4].dma_start(tl[:], wd8.ap()[idx * P:(idx + 1) * P, :])
        wd_sb.append(tl)

    # ---- Wo + residual -> xa^T (kept in f32 to the end)
    xa = []
    for cb in range(NCB):
        xa.append(pat1.tile([P, TLOC], F32, name=f"xa{cb}"))
    sps2 = pacc.tile([1, TLOC], F32, name="ssq2", tag="accA")
    for cb in range(NCB):
        for ch in range(2):
            csl = slice(ch * CHUNK, (ch + 1) * CHUNK)
            ps = pps.tile([P, CHUNK], F32, name="ops", tag="qsm", bufs=2)
            for dib in range(2):
                nc.tensor.matmul(ps[:], wo8_sb[:, dib, :, cb * P:(cb + 1) * P],
                                 yall[dib][:, :, csl],
                                 start=(dib == 0), stop=(dib == 1),
                                 perf_mode=DR)
            nc.vector.scalar_tensor_tensor(xa[cb][:, csl], ps[:], 1.0 / WSCALE,
                                           xloc[cb][:, csl],
                                           op0=ALU.mult, op1=ALU.add)
        xq = ptmp2.tile([P, TLOC], BF16, name="xsq2", tag="xsqB")
        nc.scalar.activation(xq[:], xa[cb][:], AF.Square)
        nc.tensor.matmul(sps2[:], ones_128x1[:], xq[:],
                         start=(cb == 0), stop=(cb == NCB - 1))
    if DEBUG:
        for cb in range(NCB):
            nc.sync.dma_start(dbg["d_xaT"].ap()[cb * P:(cb + 1) * P, :],
                              xa[cb][:])

    # ---- rmsnorm2 + xmoe (MoE-phase pool pmx)
    invr2 = pmx.tile([1, TLOC], F32)
    rr2 = ptmp2.tile([1, TLOC], F32, name="rms2", tag="rmsB")
    nc.scalar.activation(rr2[:], sps2[:], AF.Sqrt, bias=eps_sb[:], scale=1.0 / C)
    nc.vector.reciprocal(invr2[:], rr2[:])
    if DEBUG:
        nc.sync.dma_start(dbg["d_invr2"].ap(), invr2[:])
    bc2 = pmx.tile([P, TLOC], F32)
    nc.gpsimd.partition_broadcast(bc2[:], invr2[:])
    # normalized MoE input, fp8, DoubleRow pair layout: [:, j, :] = chan
    # block 2*dcb+j
    xmoe8 = []
    for dcb in range(4):
        tl = pmx.tile([P, 2, TLOC], F8, name=f"xmoe8_{dcb}")
        nc.gpsimd.tensor_tensor(tl[:, 0, :], xa[2 * dcb][:], bc2[:], ALU.mult)
        nc.vector.tensor_tensor(tl[:, 1, :], xa[2 * dcb + 1][:], bc2[:],
                                ALU.mult)
        xmoe8.append(tl)

    # ---- gate (fp32)
    bcomb = [pmx.tile([P, TLOC], BF16, name=f"bcomb{e}") for e in range(E)]
    for tb in range(4):
        tsl = slice(tb * P, (tb + 1) * P)
        g_ps = pps.tile([P, E], F32, name="gps", tag="sps", bufs=2)
        for cb in range(NCB):
            nc.tensor.matmul(g_ps[:], xa[cb][:, tsl], wgate_sb[:, cb, :],
                             start=(cb == 0), stop=(cb == NCB - 1))
        ir_ps = pps.tile([P, 1], F32, name="irps", tag="sps", bufs=2)
        nc.tensor.transpose(ir_ps[:], invr2[:, tsl], ones1f[:])
        ir_col = ptmp2.tile([P, 1], F32, name="ircol", tag="ircol")
        nc.scalar.copy(ir_col[:], ir_ps[:])
        lg = ptmp2.tile([P, E], F32, name="lg", tag="lg")
        nc.vector.scalar_tensor_tensor(lg[:], g_ps[:], ir_col[:], biasg_sb[:],
                                       op0=ALU.mult, op1=ALU.add)
        m1 = ptmp2.tile([P, 1], F32, name="m1", tag="m1")
        nc.vector.reduce_max(m1[:], lg[:], axis=mybir.AxisListType.X)
        eq1 = ptmp2.tile([P, E], F32, name="eq1", tag="eq1")
        nc.vector.tensor_scalar(eq1[:], lg[:], m1[:], None, op0=ALU.is_equal)
        lm = ptmp2.tile([P, E], F32, name="lm", tag="lm")
        nc.vector.scalar_tensor_tensor(lm[:], eq1[:], -1e9, lg[:],
                                       op0=ALU.mult, op1=ALU.add)
        m2 = ptmp2.tile([P, 1], F32, name="m2", tag="m2")
        nc.vector.reduce_max(m2[:], lm[:], axis=mybir.AxisListType.X)
        eq2 = ptmp2.tile([P, E], F32, name="eq2", tag="eq2")
        nc.vector.tensor_scalar(eq2[:], lm[:], m2[:], None, op0=ALU.is_equal)
        dm = ptmp2.tile([P, 1], F32, name="dm", tag="dm")
        nc.vector.tensor_scalar(dm[:], m1[:], m2[:], None, op0=ALU.subtract)
        w1 = ptmp2.tile([P, 1], F32, name="w1", tag="w1")
        nc.scalar.activation(w1[:], dm[:], AF.Sigmoid)
        w2 = ptmp2.tile([P, 1], F32, name="w2", tag="w2")
        nc.vector.tensor_scalar(w2[:], w1[:], -1.0, 1.0, op0=ALU.mult,
                                op1=ALU.add)
        cmb = ptmp2.tile([P, E], F32, name="cmb", tag="cmb")
        nc.vector.tensor_scalar(cmb[:], eq1[:], w1[:], HSCALE, op0=ALU.mult,
                                op1=ALU.mult)
        cm2 = ptmp2.tile([P, E], F32, name="cm2", tag="cm2")
        nc.vector.tensor_scalar(cm2[:], eq2[:], w2[:], HSCALE, op0=ALU.mult,
                                op1=ALU.mult)
        cmf = ptmp2.tile([P, E], F32, name="cmf", tag="cmf")
        nc.vector.tensor_tensor(cmf[:], cmb[:], cm2[:], ALU.add)
        if DEBUG:
            nc.sync.dma_start(dbg["d_comb"].ap()[:, tb * E:(tb + 1) * E],
                              cmf[:])
        ct_ps = pps.tile([E, P], F32, name="ctps", tag="sps", bufs=2)
        nc.tensor.transpose(ct_ps[:], cmf[:], ident_sb[:])
        ct_sb = ptmp2.tile([E, P], BF16, name="ctsb", tag="ctsb")
        nc.scalar.copy(ct_sb[:], ct_ps[:])
        for e in range(E):
            bc_ps = pps.tile([P, P], F32, name="bcps", tag="qsm", bufs=2)
            nc.tensor.matmul(bc_ps[:], sel8_sb[:, e * P:(e + 1) * P],
                             ct_sb[:])
            if e % 2 == 0:
                nc.scalar.copy(bcomb[e][:, tsl], bc_ps[:])
            else:
                nc.vector.tensor_scalar(bcomb[e][:, tsl], bc_ps[:], 0.0,
                                        None, op0=ALU.add)

    attn.close()

    # ---- MoE: fp8 DoubleRow matmuls, PSUM accumulation across all experts.
    # Expert 0 = shared (gating = HSCALE), experts 1..8 gated by bcomb
    # (already scaled by HSCALE).  All weights are pre-scaled by WSCALE;
    # compensations: silu scale=1/WSCALE, hh mult 1/WSCALE, final add
    # 1/(WSCALE*HSCALE).
    moe = contextlib.ExitStack()
    pw   = moe.enter_context(tc.tile_pool(name="wmoe", bufs=1))
    pgu  = moe.enter_context(tc.tile_pool(name="psG", bufs=3, space="PSUM"))
    pwd  = moe.enter_context(tc.tile_pool(name="psD", bufs=1, space="PSUM"))
    pmoe = moe.enter_context(tc.tile_pool(name="hmoe", bufs=3))
    ph8  = moe.enter_context(tc.tile_pool(name="h8p", bufs=1))

    NEXP = E + 1
    NCB_W = 5               # wd psum banks held through phase A
    h8 = [[ph8.tile([P, 2, TLOC], F8, name=f"h8_{idx}_{dib}")
           for dib in range(2)] for idx in range(NEXP)]
    wdps = [pwd.tile([P, TLOC], F32, name=f"wdps{cb}")
            for cb in range(NCB_W)]

    def wd_partial(idx, cbs, tiles):
        for cb, wt in zip(cbs, tiles):
            csl = slice(cb * P, (cb + 1) * P)
            for dib in range(2):
                nc.tensor.matmul(wt[:], wd_sb[idx][:, dib, :, csl],
                                 h8[idx][dib][:],
                                 start=(idx == 0 and dib == 0),
                                 stop=(idx == NEXP - 1 and dib == 1),
                                 perf_mode=DR)

    def expert_gu(idx):
        wg_sb = pw.tile([P, 4, 2, I], F8, name="wg8s", tag=f"wg{idx % 2}")
        nc.sync.dma_start(wg_sb[:], wg8.ap()[idx * P:(idx + 1) * P, :])
        wu_sb = pw.tile([P, 4, 2, I], F8, name="wu8s", tag=f"wu{idx % 2}")
        nc.sync.dma_start(wu_sb[:], wu8.ap()[idx * P:(idx + 1) * P, :])
        for ib in range(NIB):
            isl = slice(ib * P, (ib + 1) * P)
            gp = pgu.tile([P, TLOC], F32, name="gp", tag="psG")
            for dcb in range(4):
                nc.tensor.matmul(gp[:], wg_sb[:, dcb, :, isl], xmoe8[dcb][:],
                                 start=(dcb == 0), stop=(dcb == 3),
                                 perf_mode=DR)
            sg = pmoe.tile([P, TLOC], BF16, name="sg", tag="sg")
            nc.scalar.activation(sg[:], gp[:], AF.Silu, scale=1.0 / WSCALE)
            up = pgu.tile([P, TLOC], F32, name="up", tag="psG")
            for dcb in range(4):
                nc.tensor.matmul(up[:], wu_sb[:, dcb, :, isl], xmoe8[dcb][:],
                                 start=(dcb == 0), stop=(dcb == 3),
                                 perf_mode=DR)
            hh = pmoe.tile([P, TLOC], BF16, name="hh", tag="hh")
            nc.vector.scalar_tensor_tensor(hh[:], up[:], 1.0 / WSCALE, sg[:],
                                           op0=ALU.mult, op1=ALU.mult)
            dst = h8[idx][ib // 2][:, ib % 2, :]
            if idx == 0:
                nc.gpsimd.tensor_scalar(dst, hh[:], HSCALE, None, op0=ALU.mult)
            else:
                nc.gpsimd.tensor_tensor(dst, hh[:], bcomb[idx - 1][:],
                                        ALU.mult)

    # phase A with 1-expert-lagged partial Wd for cb < NCB_W
    for idx in range(NEXP):
        expert_gu(idx)
        if idx > 1:
            wd_partial(idx - 2, range(NCB_W), wdps)
    wd_partial(NEXP - 2, range(NCB_W), wdps)
    wd_partial(NEXP - 1, range(NCB_W), wdps)
    for cb in range(NCB_W):
        fo = pmoe.tile([P, TLOC], F32, name="fo", tag="fo")
        nc.vector.scalar_tensor_tensor(fo[:], wdps[cb][:],
                                       1.0 / (WSCALE * HSCALE),
                                       xa[cb][:], op0=ALU.mult, op1=ALU.add)
        nc.sync.dma_start(outT.ap()[cb * P:(cb + 1) * P, :], fo[:])
    # tail: remaining cb reuse the freed g/u psum slots
    for cb in range(NCB_W, NCB):
        wt = pgu.tile([P, TLOC], F32, name="wdtail", tag="psG")
        for idx in range(NEXP):
            csl = slice(cb * P, (cb + 1) * P)
            for dib in range(2):
                nc.tensor.matmul(wt[:], wd_sb[idx][:, dib, :, csl],
                                 h8[idx][dib][:],
                                 start=(idx == 0 and dib == 0),
                                 stop=(idx == NEXP - 1 and dib == 1),
                                 perf_mode=DR)
        fo = pmoe.tile([P, TLOC], F32, name="fo", tag="fo")
        nc.vector.scalar_tensor_tensor(fo[:], wt[:],
                                       1.0 / (WSCALE * HSCALE),
                                       xa[cb][:], op0=ALU.mult, op1=ALU.add)
        nc.sync.dma_start(outT.ap()[cb * P:(cb + 1) * P, :], fo[:])

    moe.close()
    whole.close()


# =============================================================== host side
def _build():
    if "nc" in _CACHE:
        return _CACHE["nc"]
    nc = bacc.Bacc("TRN2", target_bir_lowering=False, debug=False,
                   num_devices=8)
    with tile.TileContext(nc) as tc:
        _emit(nc, tc)
    nc.compile()
    _CACHE["nc"] = nc
    return nc


def _rope_tables(pos):
    # pos: (N,) positions; returns cos,ssin of shape (ROPE, N) in the
    # row-pair layout (rows 2i/2i+1 both carry angle pos*freq_i; ssin row 2i
    # is -sin, row 2i+1 is +sin).
    freqs = 1.0 / (THETA ** (np.arange(0, ROPE, 2, dtype=np.float32) / ROPE))
    ang = np.outer(freqs, pos.astype(np.float32))          # (16, N)
    cos = np.repeat(np.cos(ang), 2, axis=0).astype(np.float32)
    sin = np.sin(ang).astype(np.float32)
    ssin = np.empty((ROPE, len(pos)), np.float32)
    ssin[0::2] = -sin
    ssin[1::2] = sin
    return cos, ssin


def _host_inputs(inputs, core):
    bf = lambda a: np.ascontiguousarray(a).astype(ml_dtypes.bfloat16)
    f32 = lambda a: np.ascontiguousarray(a, dtype=np.float32)
    b, q = core // 4, core % 4
    qb_s = [15 - 4 * s - q for s in range(4)]   # query block per slot
    x = np.asarray(inputs["x"], np.float32)
    w_ln1 = np.asarray(inputs["w_ln1"], np.float32)
    w_ln2 = np.asarray(inputs["w_ln2"], np.float32)
    xT = x[b].T                                            # (C, T)
    loc_cols = np.concatenate(
        [np.arange(j * P, (j + 1) * P) for j in qb_s])
    xloc = xT[:, loc_cols]

    # rope tables
    posq = loc_cols.astype(np.float32)
    cq, sq = _rope_tables(posq)
    cosq = np.vstack([cq, cq])                             # (64, 512)
    ssinq = np.vstack([sq, sq])
    posk = np.arange(T, dtype=np.float32)
    coskp, ssinkp = _rope_tables(posk)           # (32, 2048) = [32, nt*512+t]

    # permutation matrices (pair swap)
    p32 = np.zeros((ROPE, ROPE), np.float32)
    for i in range(ROPE // 2):
        p32[2 * i + 1, 2 * i] = 1.0
        p32[2 * i, 2 * i + 1] = 1.0
    p64 = np.zeros((2 * ROPE, 2 * ROPE), np.float32)
    p64[:ROPE, :ROPE] = p32
    p64[ROPE:, ROPE:] = p32

    # causal mask for the (single) masked slot at each key block: at kb the
    # masked slot is s = 3 - kb//4 with query block qb_s[s]
    kmask = np.zeros((P, NTB * P), np.float32)
    ki = np.arange(P)[:, None]
    qi = np.arange(P)[None, :]
    for kb in range(NTB):
        j = qb_s[3 - kb // 4]
        if kb < j:
            m = np.ones((P, P), np.float32)
        elif kb == j:
            m = (ki <= qi).astype(np.float32)
        else:
            m = np.zeros((P, P), np.float32)
        kmask[:, kb * P:(kb + 1) * P] = m

    wq = np.asarray(inputs["Wq"], np.float32) * w_ln1[:, None]
    wkva = np.asarray(inputs["Wkva"], np.float32) * w_ln1[:, None]
    wo_nope = np.asarray(inputs["Wo"], np.float32).reshape(H, D, C)[:, :NOPE]
    wgate = np.asarray(inputs["Wgate"], np.float32) * w_ln2[:, None]
    biasg = np.broadcast_to(np.asarray(inputs["expert_bias"], np.float32),
                            (P, E)).copy()

    # fp8 expert weights with expert 0 = shared; DoubleRow pair packing.
    fp8 = ml_dtypes.float8_e4m3
    wg_all = np.concatenate(
        [np.asarray(inputs["sWg"], np.float32)[0:1],
         np.asarray(inputs["Wg"], np.float32)], axis=0) * w_ln2[None, :, None]
    wu_all = np.concatenate(
        [np.asarray(inputs["sWu"], np.float32)[0:1],
         np.asarray(inputs["Wu"], np.float32)], axis=0) * w_ln2[None, :, None]
    wd_all = np.concatenate(
        [np.asarray(inputs["sWd"], np.float32)[0:1],
         np.asarray(inputs["Wd"], np.float32)], axis=0)

    def pack_pairs(w, nblk):
        # w: (NEXP, K, M) with K = 256*nblk -> (NEXP*128, nblk*2*M):
        # row e*128+k, col [blk, j, m] = w[e, 256*blk + 128*j + k, m]
        ne, kk, mm = w.shape
        assert kk == 256 * nblk
        r = w.reshape(ne, nblk, 2, P, mm).transpose(0, 3, 1, 2, 4)
        return np.ascontiguousarray(r.reshape(ne * P, nblk * 2 * mm))

    fp8c = lambda a: np.clip(a, -240, 240).astype(ml_dtypes.float8_e4m3)
    m = {
        "xT8": fp8c(xT),
        "xlocT": f32(xloc),
        "xloc8": fp8c(xloc),
        "wq8": pack_pairs(wq[None] * WSCALE, 4).astype(ml_dtypes.float8_e4m3),
        "wkva8": pack_pairs(wkva[None] * WSCALE, 4).astype(ml_dtypes.float8_e4m3),
        "wkvb": bf(inputs["Wkvb"]),
        "wo8": pack_pairs(wo_nope.reshape(H * NOPE, C)[None]
                          * WSCALE, 2).astype(fp8),
        "cosq": bf(cosq), "ssinq": bf(ssinq),
        "coskp": bf(coskp), "ssinkp": bf(ssinkp),
        "perm64": bf(p64), "perm32": bf(p32),
        "ident": np.eye(P, dtype=np.float32),
        "identbf": np.eye(P, dtype=ml_dtypes.bfloat16),
        "sel8": np.kron(np.eye(E, dtype=np.float32),
                        np.ones((1, P), np.float32)).astype(ml_dtypes.bfloat16),
        "kmask": bf(kmask),
        "wgate": f32(wgate),
        "biasg": biasg,
        "wg8": pack_pairs(wg_all * WSCALE, 4).astype(fp8),
        "wu8": pack_pairs(wu_all * WSCALE, 4).astype(fp8),
        "wd8": pack_pairs(wd_all * WSCALE, 2).astype(fp8),
    }
    return m


def kernel(**inputs):
    nc = _build()
    in_maps = [_host_inputs(inputs, core) for core in range(8)]
    kw = {}
    if os.environ.get("BASSK_TRACE"):
        kw = dict(trace=True, tmpdir=os.environ.get("BASSK_TRACE_DIR") or None)
    res = bass_utils.run_bass_kernel_spmd(nc, in_maps, core_ids=list(range(8)),
                                          **kw)
    if os.environ.get("BASSK_TRACE"):
        print(f"PROFILE exec_time_ns: {res.exec_time_ns}")
        print(f"PROFILE json: {res.profile_json}")
    out = np.empty((B, T, C), np.float32)
    for core in range(8):
        b, q = core // 4, core % 4
        oT = res.results[core]["outT"]                     # (C, 512)
        for s in range(4):
            j = 15 - 4 * s - q
            out[b, j * P:(j + 1) * P] = oT[:, s * P:(s + 1) * P].T
    return out

